# revision 18
# baseline (speedup 1.0000x reference)
"""Trainium2 Bass kernel for GQA attention (B=2, S=2048, D=2048, H=32, KVH=8).

Sharding: batch data-parallel across 2 groups of 4 cores; within a group,
4-way tensor parallel over heads (8 q heads + their 2 kv heads per core).
Device-side per-chunk ReduceScatter(add) over each 4-core group after the wo
matmul; the host concatenates the token slices.

The device program is identical on all 8 cores (SPMD); all per-core
variation (batch slice, head slice) is carried by the input data.

v2 structure: a software pipeline over 512-token chunks —
   proj(c) -> rope(c) -> kTrep/v(c) -> attention(c) -> wo(c) -> RS(c)
so the tensor engine stays dense (HAM stays warm) and the collective
overlaps compute. Projections and the wo matmul run in fp32r; the
attention path (q/k/v/probs/cos/mask) runs in bf16 (fp32 PSUM accum).

Layout notes:
 - Host passes x pre-transposed (xT, chunk-major); every matmul consumes xT
   directly (no on-device transposes of activations).
 - wq/wk columns are permuted on host into an "even dims block / odd dims
   block" (A/B) layout so RoPE is full-partition DVE work; wq carries the
   1/sqrt(HD) scale (exact power of two).
 - Scores are computed transposed (scoresT[sk, sq]) so probsT feeds the AV
   matmul directly with no transposes in the attention path.
 - Causal mask: one extra accumulating matmul per diagonal tile,
   LEones[k,p]=[k<=p] x shifted -1e9 diagonal, adds -1e9 to masked entries.
 - Softmax denominators ride along as a ones column in v (M=65 AV matmul);
   normalization multiplies by the partition-broadcast reciprocal.
"""

import os
import sys
import functools

import numpy as np

if "/opt/trn_rl_repo" not in sys.path:
    sys.path.insert(0, "/opt/trn_rl_repo")

B, S, D = 2, 2048, 2048
H, KVH = 32, 8
HD = D // H            # 64
N_CORES = 8
GROUP = 4              # cores per batch group (tensor parallel width)
HPC = 8                # query heads per core
KVPC = 2               # kv heads per core
SQC = 512              # sq chunk (psum bank width in fp32)
PT = 128               # partition tile
KT = D // PT           # 16 contraction tiles
NT = S // PT           # 16 token tiles
NCHUNK = S // SQC      # 4
TPC = SQC // PT        # tok tiles per chunk (4)
MASK_W = 896           # width of the shifted-diagonal mask table
NEG = -1e9


def _build_program(mm_dtype_name="float32r"):
    import concourse.bass as bass
    import concourse.bacc as bacc
    import concourse.mybir as mybir
    import concourse.tile as tile
    import ml_dtypes
    from contextlib import ExitStack

    f32 = mybir.dt.float32
    bf16 = mybir.dt.bfloat16
    mmdt = getattr(mybir.dt, mm_dtype_name)

    def mc(ap):  # bitcast for fp32r matmul operands/producers
        return ap.bitcast(mmdt) if mmdt != f32 else ap

    nc = bacc.Bacc("TRN2", target_bir_lowering=False, debug=False,
                   num_devices=N_CORES)

    # ---- dram parameters -------------------------------------------------
    xT_d = nc.dram_tensor("xt", [NCHUNK, D, SQC], bf16, kind="ExternalInput")
    wq_d = nc.dram_tensor("wq", [D, HPC * HD], bf16, kind="ExternalInput")
    wk_d = nc.dram_tensor("wk", [D, KVPC * HD], bf16, kind="ExternalInput")
    wv_d = nc.dram_tensor("wv", [D, KVPC * HD], bf16, kind="ExternalInput")
    wo_d = nc.dram_tensor("wo", [HPC * HD, D], f32, kind="ExternalInput")
    cos_d = nc.dram_tensor("cosr", [PT, S], bf16, kind="ExternalInput")
    sin_d = nc.dram_tensor("sinr", [PT, S], bf16, kind="ExternalInput")
    y_out = nc.dram_tensor("y", [S // GROUP, D], f32, kind="ExternalOutput")

    y_part = nc.dram_tensor("y_part", [S, D], f32)
    y_rs = nc.dram_tensor("y_rs", [S // GROUP, D], f32)

    # ---- inline constants ------------------------------------------------
    leones = np.zeros((PT, PT), np.float32)      # leones[k, p] = 1 if k <= p
    for k in range(PT):
        leones[k, k:] = 1.0
    dmaster = np.zeros((PT, MASK_W), np.float32)
    # col m: -1e9 one-hot at k = m-383 for m in [384, 510]; row 0 for m<384
    dmaster[0, :384] = NEG
    for m in range(384, 511):
        dmaster[m - 383, m] = NEG
    ident = np.eye(PT, dtype=ml_dtypes.bfloat16)
    ones1 = np.ones((PT, 1), ml_dtypes.bfloat16)

    le_d = nc.inline_tensor(leones.astype(ml_dtypes.bfloat16), "leones")
    dm_d = nc.inline_tensor(dmaster.astype(ml_dtypes.bfloat16), "dmaster")
    id_d = nc.inline_tensor(ident, "ident")
    on_d = nc.inline_tensor(ones1, "ones1")

    Exp = mybir.ActivationFunctionType.Exp
    groups = [[0, 1, 2, 3], [4, 5, 6, 7]]

    with tile.TileContext(nc) as tc, ExitStack() as ctx:
        keep = ctx.enter_context(tc.tile_pool(name="keep", bufs=1))
        kTrep = keep.tile([PT, 4, S], bf16)    # Akv0 Akv1 Bkv0 Bkv1 (x4 rows)
        v_sb = keep.tile([PT, KVPC, NT, HD + 1], bf16)   # col 64 = ones
        cos_sb = keep.tile([PT, S], bf16)
        sin_sb = keep.tile([PT, S], bf16)
        le_sb = keep.tile([PT, PT], bf16)
        dm_sb = keep.tile([PT, MASK_W], bf16)
        id_sb = keep.tile([PT, PT], bf16)
        wq_sb = keep.tile([PT, KT, HPC * HD], bf16)
        wk_sb = keep.tile([PT, KT, KVPC * HD], bf16)
        wv_sb = keep.tile([PT, KT, KVPC * HD], bf16)
        wo_sb = keep.tile([PT, 4, D], f32)

        nc.sync.dma_start(out=le_sb[:], in_=le_d[:])
        nc.sync.dma_start(out=dm_sb[:], in_=dm_d[:])
        nc.sync.dma_start(out=id_sb[:], in_=id_d[:])
        nc.sync.dma_start(out=cos_sb[:], in_=cos_d[:])
        nc.sync.dma_start(out=sin_sb[:], in_=sin_d[:])
        nc.sync.dma_start(out=wq_sb[:],
                          in_=wq_d.ap().rearrange("(k p) n -> p k n", p=PT))
        nc.sync.dma_start(out=wk_sb[:],
                          in_=wk_d.ap().rearrange("(k p) n -> p k n", p=PT))
        nc.sync.dma_start(out=wv_sb[:],
                          in_=wv_d.ap().rearrange("(k p) n -> p k n", p=PT))
        nc.sync.dma_start(out=mc(wo_sb[:]),
                          in_=mc(wo_d.ap().rearrange("(k p) n -> p k n", p=PT)))
        # ones column of v (every (kv, t) slot)
        ones_src = bass.AP(tensor=on_d.ap().tensor, offset=0,
                           ap=[[1, PT], [0, KVPC * NT], [1, 1]])
        vcol = v_sb[:, :, :, HD:HD + 1]
        ones_dst = bass.AP(tensor=vcol.tensor, offset=vcol.offset,
                           ap=[list(vcol.ap[0]), [HD + 1, KVPC * NT], [1, 1]])
        nc.sync.dma_start(out=ones_dst, in_=ones_src)

        xpool = ctx.enter_context(tc.tile_pool(name="xp", bufs=3))
        qpool = ctx.enter_context(tc.tile_pool(name="qp", bufs=2))
        kpool = ctx.enter_context(tc.tile_pool(name="kp", bufs=2))
        vtp = ctx.enter_context(tc.tile_pool(name="vtp", bufs=2))
        otp = ctx.enter_context(tc.tile_pool(name="otp", bufs=2))
        rtmp = ctx.enter_context(tc.tile_pool(name="rtmp", bufs=1))
        probs = ctx.enter_context(tc.tile_pool(name="probs", bufs=8))
        bcp = ctx.enter_context(tc.tile_pool(name="bcp", bufs=2))
        rcp = ctx.enter_context(tc.tile_pool(name="rcp", bufs=2))
        osg = ctx.enter_context(tc.tile_pool(name="osg", bufs=2))
        ysb = ctx.enter_context(tc.tile_pool(name="ysb", bufs=3))
        mw = ctx.enter_context(tc.tile_pool(name="mw", bufs=2, space="PSUM"))
        sps = ctx.enter_context(tc.tile_pool(name="sps", bufs=4, space="PSUM"))
        aps = ctx.enter_context(tc.tile_pool(name="aps", bufs=2, space="PSUM"))

        def rope_pair(a, b, cs, sn, nm):
            """a' = a*cos - b*sin ; b' = a*sin + b*cos (bf16, in place)."""
            t1 = rtmp.tile(a.shape, bf16, tag="t1", name=f"t1{nm}")
            t2 = rtmp.tile(a.shape, bf16, tag="t2", name=f"t2{nm}")
            t3 = rtmp.tile(a.shape, bf16, tag="t3", name=f"t3{nm}")
            nc.vector.tensor_mul(t1[:], a, cs)
            nc.vector.tensor_mul(t2[:], a, sn)
            nc.vector.tensor_mul(t3[:], b, sn)
            nc.vector.tensor_sub(a, t1[:], t3[:])
            t4 = rtmp.tile(a.shape, bf16, tag="t3", name=f"t4{nm}")
            nc.vector.tensor_mul(t4[:], b, cs)
            nc.vector.tensor_add(b, t2[:], t4[:])

        for c in range(NCHUNK):
            csl = slice(c * SQC, (c + 1) * SQC)

            # ---- proj(c): qT chunk (bf16), kT chunk (bf16), vT chunk ----
            halves = []
            for hf in range(2):
                xt = xpool.tile([PT, KT // 2, SQC], bf16, tag="xt",
                                name=f"xt{c}_{hf}")
                nc.sync.dma_start(
                    out=xt[:],
                    in_=xT_d[c].rearrange("(k p) n -> p k n", p=PT)
                    [:, hf * (KT // 2):(hf + 1) * (KT // 2), :])
                halves.append(xt)

            qc = qpool.tile([PT, 4, SQC], bf16, tag="qc", name=f"qc{c}")
            kc = kpool.tile([PT, SQC], bf16, tag="kc", name=f"kc{c}")
            vtc = vtp.tile([PT, SQC], bf16, tag="vtc", name=f"vtc{c}")
            for mt in range(4):
                ps = mw.tile([PT, SQC], f32, tag="ps", name=f"qps{c}_{mt}")
                for k in range(KT):
                    nc.tensor.matmul(
                        ps[:], wq_sb[:, k, mt * PT:(mt + 1) * PT],
                        halves[k // 8][:, k % 8, :],
                        start=(k == 0), stop=(k == KT - 1))
                nc.scalar.copy(qc[:, mt, :], ps[:])
            for dst, wsb, nm in ((kc, wk_sb, "k"), (vtc, wv_sb, "v")):
                ps = mw.tile([PT, SQC], f32, tag="ps", name=f"ps{nm}{c}")
                for k in range(KT):
                    nc.tensor.matmul(
                        ps[:], wsb[:, k, :],
                        halves[k // 8][:, k % 8, :],
                        start=(k == 0), stop=(k == KT - 1))
                nc.scalar.copy(dst[:], ps[:])

            # ---- rope(c) ------------------------------------------------
            for j in range(2):
                rope_pair(qc[:, j, :], qc[:, 2 + j, :],
                          cos_sb[:, csl], sin_sb[:, csl], f"q{c}_{j}")
            # k pair: rows 0:64 / 64:128 — stage B rows to base 0 via DMA
            bst = rtmp.tile([64, SQC], bf16, tag="t1", name=f"bst{c}")
            nc.sync.dma_start(out=bst[:], in_=kc[64:128, :])
            kt1 = rtmp.tile([64, SQC], bf16, tag="t2", name=f"kt1{c}")
            kt2 = rtmp.tile([64, SQC], bf16, tag="t3", name=f"kt2{c}")
            kt3 = rtmp.tile([64, SQC], bf16, tag="t1b", name=f"kt3{c}")
            kt4 = rtmp.tile([64, SQC], bf16, tag="t2b", name=f"kt4{c}")
            nc.vector.tensor_mul(kt1[:], kc[0:64, :], cos_sb[0:64, csl])
            nc.vector.tensor_mul(kt2[:], kc[0:64, :], sin_sb[0:64, csl])
            nc.vector.tensor_mul(kt3[:], bst[:], sin_sb[0:64, csl])
            nc.vector.tensor_mul(kt4[:], bst[:], cos_sb[0:64, csl])
            nc.vector.tensor_sub(kc[0:64, :], kt1[:], kt3[:])
            kbr = rtmp.tile([64, SQC], bf16, tag="t3b", name=f"kbr{c}")
            nc.vector.tensor_add(kbr[:], kt2[:], kt4[:])
            nc.sync.dma_start(out=kc[64:128, :], in_=kbr[:])

            # ---- kTrep(c): each 32-row group replicated x4 --------------
            for r in range(4):
                for slot in range(4):
                    nc.sync.dma_start(
                        out=kTrep[slot * 32:(slot + 1) * 32, r, csl],
                        in_=kc[r * 32:(r + 1) * 32, :])

            # ---- v(c): transpose vT chunk into v_sb ---------------------
            for tl in range(TPC):
                t = c * TPC + tl
                tp = mw.tile([PT, SQC], f32, tag="ps", name=f"tp{c}_{tl}")
                tpb = tp[:, 0:PT].bitcast(bf16)[:, 0:PT]
                nc.tensor.transpose(tpb,
                                    vtc[:, tl * PT:(tl + 1) * PT],
                                    id_sb[:])
                nc.vector.tensor_copy(v_sb[:, 0, t, 0:HD], tpb[:, 0:HD])
                nc.vector.tensor_copy(v_sb[:, 1, t, 0:HD], tpb[:, HD:2 * HD])

            # ---- attention(c) -------------------------------------------
            outc = otp.tile([PT, 4, SQC], f32, tag="outc", name=f"outc{c}")
            ntk = 4 * c + 4
            for g in range(2):
                for m in range(4):
                    msl = slice(m * 32, (m + 1) * 32)
                    av = aps.tile([PT, SQC], f32, tag="av",
                                  name=f"av{c}_{g}_{m}")
                    for t in range(ntk):
                        ksl = slice(t * PT, (t + 1) * PT)
                        sc = sps.tile([PT, SQC], f32, tag="sc",
                                      name=f"sc{c}_{g}_{m}_{t}")
                        diag = t >= 4 * c
                        nc.tensor.matmul(
                            sc[:], kTrep[msl, g, ksl], qc[msl, g, :],
                            start=True, stop=False,
                            tile_position=(m * 32, 0))
                        nc.tensor.matmul(
                            sc[:], kTrep[msl, 2 + g, ksl],
                            qc[msl, 2 + g, :],
                            start=False, stop=not diag,
                            tile_position=(m * 32, 0))
                        if diag:
                            r = t - 4 * c
                            nc.tensor.matmul(
                                sc[:], le_sb[:],
                                dm_sb[:, 384 - 128 * r:MASK_W - 128 * r],
                                start=False, stop=True)
                        pb = probs.tile([PT, SQC], bf16, tag="pb",
                                        name=f"pb{c}_{g}_{m}_{t}")
                        nc.scalar.activation(pb[:], sc[:], Exp)
                        nc.tensor.matmul(
                            av[0:HD + 1, :], v_sb[:, g, t, :], pb[:],
                            start=(t == 0), stop=(t == ntk - 1))
                    qh = g * 4 + m
                    rc = rcp.tile([1, SQC], f32, tag="rc",
                                  name=f"rc{c}_{g}_{m}")
                    nc.vector.reciprocal(rc[:], av[HD:HD + 1, :])
                    bc = bcp.tile([64, SQC], f32, tag="bc",
                                  name=f"bc{c}_{g}_{m}")
                    nc.gpsimd.partition_broadcast(bc[:], rc[:])
                    dst = outc[(qh % 2) * HD:(qh % 2 + 1) * HD, qh // 2, :]
                    if qh % 2 == 0:
                        nc.vector.tensor_mul(mc(dst), av[0:HD, :], bc[:])
                    else:
                        st = osg.tile([64, SQC], f32, tag="st",
                                      name=f"st{c}_{g}_{m}")
                        nc.vector.tensor_mul(st[:], av[0:HD, :], bc[:])
                        nc.sync.dma_start(out=mc(dst), in_=mc(st[:]))

            # ---- wo(c) --------------------------------------------------
            for tl in range(TPC):
                tt = c * TPC + tl
                yt = ysb.tile([PT, D], f32, tag="yt", name=f"yt{c}_{tl}")
                for nk in range(4):
                    yp = mw.tile([PT, SQC], f32, tag="ps",
                                 name=f"yp{c}_{tl}_{nk}")
                    for k4 in range(4):
                        nc.tensor.matmul(
                            yp[:], mc(outc[:, k4, tl * PT:(tl + 1) * PT]),
                            mc(wo_sb[:, k4, nk * SQC:(nk + 1) * SQC]),
                            start=(k4 == 0), stop=(k4 == 3))
                    nc.vector.tensor_copy(yt[:, nk * SQC:(nk + 1) * SQC],
                                          yp[:])
                nc.sync.dma_start(out=y_part[tt * PT:(tt + 1) * PT, :],
                                  in_=yt[:])

            # ---- RS(c-1): issue the previous chunk's reduce-scatter -----
            # (delayed one chunk so peer-skew waits on the gpsimd queue
            #  never block the next chunk's partition_broadcasts)
            if c > 0:
                pc = c - 1
                nc.gpsimd.collective_compute(
                    "ReduceScatter", mybir.AluOpType.add,
                    replica_groups=groups,
                    ins=[y_part.ap()[pc * SQC:(pc + 1) * SQC, :]],
                    outs=[y_rs.ap()[pc * PT:(pc + 1) * PT, :]])
                nc.sync.dma_start(
                    out=y_out.ap()[pc * PT:(pc + 1) * PT, :],
                    in_=y_rs.ap()[pc * PT:(pc + 1) * PT, :])

        pc = NCHUNK - 1
        nc.gpsimd.collective_compute(
            "ReduceScatter", mybir.AluOpType.add, replica_groups=groups,
            ins=[y_part.ap()[pc * SQC:(pc + 1) * SQC, :]],
            outs=[y_rs.ap()[pc * PT:(pc + 1) * PT, :]])
        nc.sync.dma_start(out=y_out.ap()[pc * PT:(pc + 1) * PT, :],
                          in_=y_rs.ap()[pc * PT:(pc + 1) * PT, :])

    nc.compile()
    return nc


@functools.lru_cache(maxsize=2)
def _get_program(mm_dtype_name="float32r"):
    return _build_program(mm_dtype_name)


def _host_inputs(x, wq, wk, wv, wo, cos, sin):
    """Build the 8 per-core input maps."""
    import ml_dtypes

    perm_q = np.empty(HPC * HD, np.int64)
    for rho in range(HPC * HD):
        blk, rem = divmod(rho, HPC * HD // 2)
        h, i = divmod(rem, 32)
        perm_q[rho] = h * HD + 2 * i + blk
    perm_k = np.empty(KVPC * HD, np.int64)
    for rho in range(KVPC * HD):
        blk, rem = divmod(rho, KVPC * HD // 2)
        kv, i = divmod(rem, 32)
        perm_k[rho] = kv * HD + 2 * i + blk

    reps = np.tile(np.arange(32), 4)
    cosr = np.ascontiguousarray(cos.T[reps]).astype(ml_dtypes.bfloat16)
    sinr = np.ascontiguousarray(sin.T[reps]).astype(ml_dtypes.bfloat16)

    xts = []
    for b in range(B):
        xt = x[b].T.reshape(D, NCHUNK, SQC)       # [D, 4, 512]
        xts.append(np.ascontiguousarray(xt.transpose(1, 0, 2))
                   .astype(ml_dtypes.bfloat16))

    scale = np.float32(1.0 / np.sqrt(HD))
    in_maps = []
    for core in range(N_CORES):
        b, hg = divmod(core, GROUP)
        qcols = slice(hg * HPC * HD, (hg + 1) * HPC * HD)
        kcols = slice(hg * KVPC * HD, (hg + 1) * KVPC * HD)
        wq_c = (wq[:, qcols] * scale)[:, perm_q].astype(ml_dtypes.bfloat16)
        wk_c = wk[:, kcols][:, perm_k].astype(ml_dtypes.bfloat16)
        wv_c = np.ascontiguousarray(wv[:, kcols]).astype(ml_dtypes.bfloat16)
        wo_c = np.ascontiguousarray(wo[qcols, :])
        in_maps.append({
            "xt": xts[b],
            "wq": np.ascontiguousarray(wq_c),
            "wk": np.ascontiguousarray(wk_c),
            "wv": wv_c,
            "wo": wo_c,
            "cosr": cosr,
            "sinr": sinr,
        })
    return in_maps


def _assemble(results):
    """results[core]["y"] rows are [chunk(4) x 128] token blocks."""
    out = np.empty((B, S, D), np.float32)
    for b in range(B):
        for r in range(GROUP):
            y = results[b * GROUP + r]["y"]
            for c in range(NCHUNK):
                rows = slice(c * SQC + r * PT, c * SQC + (r + 1) * PT)
                out[b, rows, :] = y[c * PT:(c + 1) * PT, :]
    return out


def _is_causal(mask):
    if mask.shape != (S, S):
        return False
    expect = np.where(np.tril(np.ones((S, S), bool)), np.float32(0.0),
                      np.float32(NEG))
    return np.array_equal(mask, expect)


def _numpy_fallback(x, wq, wk, wv, wo, cos, sin, mask):
    """Exact reference math on host (only used if mask isn't causal)."""
    xq = (x @ wq).reshape(B, S, H, HD)
    xk = (x @ wk).reshape(B, S, KVH, HD)
    xv = (x @ wv).reshape(B, S, KVH, HD)

    def rope(t):
        tr = t.reshape(*t.shape[:-1], HD // 2, 2)
        a, b = tr[..., 0], tr[..., 1]
        c = cos[None, :, None, :]
        s_ = sin[None, :, None, :]
        out = np.stack([a * c - b * s_, a * s_ + b * c], axis=-1)
        return out.reshape(t.shape)

    xq, xk = rope(xq), rope(xk)
    xk = np.repeat(xk, H // KVH, axis=2)
    xv = np.repeat(xv, H // KVH, axis=2)
    q = xq.transpose(0, 2, 1, 3)
    k = xk.transpose(0, 2, 1, 3)
    v = xv.transpose(0, 2, 1, 3)
    sc = np.einsum("bhqd,bhkd->bhqk", q, k) / np.sqrt(np.float32(HD))
    sc = sc + mask[None, None]
    sc = sc - sc.max(-1, keepdims=True)
    p = np.exp(sc)
    p /= p.sum(-1, keepdims=True)
    out = np.einsum("bhqk,bhkd->bhqd", p, v)
    out = out.transpose(0, 2, 1, 3).reshape(B, S, H * HD)
    return (out @ wo).astype(np.float32)


def _ensure_ntff_hook():
    """Provide antenv.axon_hooks (missing on this image) so trace=True works."""
    try:
        from antenv.axon_hooks import get_axon_ntff_profile_hook  # noqa: F401
        return True
    except ImportError:
        pass
    try:
        import types
        import antenv
        from trn_agent_boot.trn_boot import _ntff_profile_via_ctypes

        mod = types.ModuleType("antenv.axon_hooks")
        _state = {"hook": None}
        mod.set_axon_ntff_profile_hook = \
            lambda h: _state.__setitem__("hook", h)
        mod.get_axon_ntff_profile_hook = lambda: _state["hook"]
        sys.modules["antenv.axon_hooks"] = mod
        antenv.axon_hooks = mod
        mod.set_axon_ntff_profile_hook(
            _ntff_profile_via_ctypes("/opt/axon/libaxon_pjrt.so"))
        return mod.get_axon_ntff_profile_hook() is not None
    except Exception:
        return False


def kernel(x, wq, wk, wv, wo, cos, sin, mask):
    x = np.asarray(x, np.float32)
    wq = np.asarray(wq, np.float32)
    wk = np.asarray(wk, np.float32)
    wv = np.asarray(wv, np.float32)
    wo = np.asarray(wo, np.float32)
    cos = np.asarray(cos, np.float32)
    sin = np.asarray(sin, np.float32)
    mask = np.asarray(mask, np.float32)

    if not _is_causal(mask):
        return _numpy_fallback(x, wq, wk, wv, wo, cos, sin, mask)

    from concourse.bass_utils import run_bass_kernel_spmd

    nc = _get_program(os.environ.get("ATTN_MM_DTYPE", "float32r"))
    in_maps = _host_inputs(x, wq, wk, wv, wo, cos, sin)
    trace = bool(int(os.environ.get("ATTN_TRACE", "0")))
    if trace and not _ensure_ntff_hook():
        trace = False
    res = run_bass_kernel_spmd(nc, in_maps, core_ids=list(range(N_CORES)),
                               trace=trace)
    if trace:
        kernel.last_exec_time_ns = res.exec_time_ns
        kernel.last_results = res
    return _assemble(res.results)


# revision 21
# speedup vs baseline: 1.0604x; 1.0604x over previous
"""Trainium2 Bass kernel for GQA attention (B=2, S=2048, D=2048, H=32, KVH=8).

Sharding: batch data-parallel across 2 groups of 4 cores; within a group,
4-way tensor parallel over heads (8 q heads + their 2 kv heads per core).
Device-side per-chunk ReduceScatter(add) over each 4-core group after the wo
matmul; the host concatenates the token slices.

The device program is identical on all 8 cores (SPMD); all per-core
variation (batch slice, head slice) is carried by the input data.

v2 structure: a software pipeline over 512-token chunks —
   proj(c) -> rope(c) -> kTrep/v(c) -> attention(c) -> wo(c) -> RS(c)
so the tensor engine stays dense (HAM stays warm) and the collective
overlaps compute. Projections and the wo matmul run in fp32r; the
attention path (q/k/v/probs/cos/mask) runs in bf16 (fp32 PSUM accum).

Layout notes:
 - Host passes x pre-transposed (xT, chunk-major); every matmul consumes xT
   directly (no on-device transposes of activations).
 - wq/wk columns are permuted on host into an "even dims block / odd dims
   block" (A/B) layout so RoPE is full-partition DVE work; wq carries the
   1/sqrt(HD) scale (exact power of two).
 - Scores are computed transposed (scoresT[sk, sq]) so probsT feeds the AV
   matmul directly with no transposes in the attention path.
 - Causal mask: one extra accumulating matmul per diagonal tile,
   LEones[k,p]=[k<=p] x shifted -1e9 diagonal, adds -1e9 to masked entries.
 - Softmax denominators ride along as a ones column in v (M=65 AV matmul);
   normalization multiplies by the partition-broadcast reciprocal.
"""

import os
import sys
import functools

import numpy as np

if "/opt/trn_rl_repo" not in sys.path:
    sys.path.insert(0, "/opt/trn_rl_repo")

B, S, D = 2, 2048, 2048
H, KVH = 32, 8
HD = D // H            # 64
N_CORES = 8
GROUP = 4              # cores per batch group (tensor parallel width)
HPC = 8                # query heads per core
KVPC = 2               # kv heads per core
SQC = 512              # sq chunk (psum bank width in fp32)
PT = 128               # partition tile
KT = D // PT           # 16 contraction tiles
NT = S // PT           # 16 token tiles
NCHUNK = S // SQC      # 4
TPC = SQC // PT        # tok tiles per chunk (4)
MASK_W = 896           # width of the shifted-diagonal mask table
NEG = -1e9


def _build_program(mm_dtype_name="float32r"):
    import concourse.bass as bass
    import concourse.bacc as bacc
    import concourse.mybir as mybir
    import concourse.tile as tile
    import ml_dtypes
    from contextlib import ExitStack

    f32 = mybir.dt.float32
    bf16 = mybir.dt.bfloat16
    mmdt = getattr(mybir.dt, mm_dtype_name)

    def mc(ap):  # bitcast for fp32r matmul operands/producers
        return ap.bitcast(mmdt) if mmdt != f32 else ap

    nc = bacc.Bacc("TRN2", target_bir_lowering=False, debug=False,
                   num_devices=N_CORES)

    # ---- dram parameters -------------------------------------------------
    xT_d = nc.dram_tensor("xt", [NCHUNK, D, SQC], bf16, kind="ExternalInput")
    wq_d = nc.dram_tensor("wq", [D, HPC * HD], bf16, kind="ExternalInput")
    wk_d = nc.dram_tensor("wk", [D, KVPC * HD], bf16, kind="ExternalInput")
    wv_d = nc.dram_tensor("wv", [D, KVPC * HD], bf16, kind="ExternalInput")
    wo_d = nc.dram_tensor("wo", [HPC * HD, D], f32, kind="ExternalInput")
    cos_d = nc.dram_tensor("cosr", [PT, S], bf16, kind="ExternalInput")
    sin_d = nc.dram_tensor("sinr", [PT, S], bf16, kind="ExternalInput")
    y_out = nc.dram_tensor("y", [S // GROUP, D], f32, kind="ExternalOutput")

    y_part = nc.dram_tensor("y_part", [S, D], f32)
    y_rs = nc.dram_tensor("y_rs", [S // GROUP, D], f32)

    # ---- inline constants ------------------------------------------------
    leones = np.zeros((PT, PT), np.float32)      # leones[k, p] = 1 if k <= p
    for k in range(PT):
        leones[k, k:] = 1.0
    dmaster = np.zeros((PT, MASK_W), np.float32)
    # col m: -1e9 one-hot at k = m-383 for m in [384, 510]; row 0 for m<384
    dmaster[0, :384] = NEG
    for m in range(384, 511):
        dmaster[m - 383, m] = NEG
    ident = np.eye(PT, dtype=ml_dtypes.bfloat16)
    ones1 = np.ones((PT, 1), ml_dtypes.bfloat16)

    le_d = nc.inline_tensor(leones.astype(ml_dtypes.bfloat16), "leones")
    dm_d = nc.inline_tensor(dmaster.astype(ml_dtypes.bfloat16), "dmaster")
    id_d = nc.inline_tensor(ident, "ident")
    on_d = nc.inline_tensor(ones1, "ones1")

    Exp = mybir.ActivationFunctionType.Exp
    groups = [[0, 1, 2, 3], [4, 5, 6, 7]]

    with tile.TileContext(nc) as tc, ExitStack() as ctx:
        keep = ctx.enter_context(tc.tile_pool(name="keep", bufs=1))
        kTrep = keep.tile([PT, 4, S], bf16)    # Akv0 Akv1 Bkv0 Bkv1 (x4 rows)
        v_sb = keep.tile([PT, KVPC, NT, HD + 1], bf16)   # col 64 = ones
        cos_sb = keep.tile([PT, S], bf16)
        sin_sb = keep.tile([PT, S], bf16)
        le_sb = keep.tile([PT, PT], bf16)
        dm_sb = keep.tile([PT, MASK_W], bf16)
        id_sb = keep.tile([PT, PT], bf16)
        wq_sb = keep.tile([PT, KT, HPC * HD], bf16)
        wk_sb = keep.tile([PT, KT, KVPC * HD], bf16)
        wv_sb = keep.tile([PT, KT, KVPC * HD], bf16)
        wo_sb = keep.tile([PT, 4, D], f32)

        xcache = {}

        def load_x(c):
            halves = []
            for hf in range(2):
                xt = xpool.tile([PT, KT // 2, SQC], bf16, tag="xt",
                                name=f"xt{c}_{hf}")
                nc.sync.dma_start(
                    out=xt[:],
                    in_=xT_d[c].rearrange("(k p) n -> p k n", p=PT)
                    [:, hf * (KT // 2):(hf + 1) * (KT // 2), :])
                halves.append(xt)
            xcache[c] = halves

        nc.sync.dma_start(out=wq_sb[:],
                          in_=wq_d.ap().rearrange("(k p) n -> p k n", p=PT))
        nc.sync.dma_start(out=wk_sb[:],
                          in_=wk_d.ap().rearrange("(k p) n -> p k n", p=PT))
        nc.sync.dma_start(out=wv_sb[:],
                          in_=wv_d.ap().rearrange("(k p) n -> p k n", p=PT))
        nc.sync.dma_start(out=cos_sb[:], in_=cos_d[:])
        nc.sync.dma_start(out=sin_sb[:], in_=sin_d[:])
        nc.sync.dma_start(out=le_sb[:], in_=le_d[:])
        nc.sync.dma_start(out=dm_sb[:], in_=dm_d[:])
        nc.sync.dma_start(out=id_sb[:], in_=id_d[:])
        nc.sync.dma_start(out=mc(wo_sb[:]),
                          in_=mc(wo_d.ap().rearrange("(k p) n -> p k n", p=PT)))
        # ones column of v (every (kv, t) slot)
        ones_src = bass.AP(tensor=on_d.ap().tensor, offset=0,
                           ap=[[1, PT], [0, KVPC * NT], [1, 1]])
        vcol = v_sb[:, :, :, HD:HD + 1]
        ones_dst = bass.AP(tensor=vcol.tensor, offset=vcol.offset,
                           ap=[list(vcol.ap[0]), [HD + 1, KVPC * NT], [1, 1]])
        nc.sync.dma_start(out=ones_dst, in_=ones_src)

        xpool = ctx.enter_context(tc.tile_pool(name="xp", bufs=4))
        qpool = ctx.enter_context(tc.tile_pool(name="qp", bufs=2))
        kpool = ctx.enter_context(tc.tile_pool(name="kp", bufs=2))
        vtp = ctx.enter_context(tc.tile_pool(name="vtp", bufs=2))
        otp = ctx.enter_context(tc.tile_pool(name="otp", bufs=2))
        rtmp = ctx.enter_context(tc.tile_pool(name="rtmp", bufs=1))
        probs = ctx.enter_context(tc.tile_pool(name="probs", bufs=8))
        bcp = ctx.enter_context(tc.tile_pool(name="bcp", bufs=2))
        rcp = ctx.enter_context(tc.tile_pool(name="rcp", bufs=2))
        osg = ctx.enter_context(tc.tile_pool(name="osg", bufs=2))
        ysb = ctx.enter_context(tc.tile_pool(name="ysb", bufs=3))
        mw = ctx.enter_context(tc.tile_pool(name="mw", bufs=2, space="PSUM"))
        sps = ctx.enter_context(tc.tile_pool(name="sps", bufs=4, space="PSUM"))
        aps = ctx.enter_context(tc.tile_pool(name="aps", bufs=2, space="PSUM"))

        load_x(0)

        def rope_pair(a, b, cs, sn, nm):
            """a' = a*cos - b*sin ; b' = a*sin + b*cos (bf16, in place)."""
            t1 = rtmp.tile(a.shape, bf16, tag="t1", name=f"t1{nm}")
            t2 = rtmp.tile(a.shape, bf16, tag="t2", name=f"t2{nm}")
            t3 = rtmp.tile(a.shape, bf16, tag="t3", name=f"t3{nm}")
            nc.vector.tensor_mul(t1[:], a, cs)
            nc.vector.tensor_mul(t2[:], a, sn)
            nc.vector.tensor_mul(t3[:], b, sn)
            nc.vector.tensor_sub(a, t1[:], t3[:])
            t4 = rtmp.tile(a.shape, bf16, tag="t3", name=f"t4{nm}")
            nc.vector.tensor_mul(t4[:], b, cs)
            nc.vector.tensor_add(b, t2[:], t4[:])

        for c in range(NCHUNK):
            csl = slice(c * SQC, (c + 1) * SQC)

            # ---- proj(c): qT chunk (bf16), kT chunk (bf16), vT chunk ----
            if c not in xcache:
                load_x(c)
            halves = xcache.pop(c)

            qc = qpool.tile([PT, 4, SQC], bf16, tag="qc", name=f"qc{c}")
            kc = kpool.tile([PT, SQC], bf16, tag="kc", name=f"kc{c}")
            vtc = vtp.tile([PT, SQC], bf16, tag="vtc", name=f"vtc{c}")
            for mt in range(4):
                ps = mw.tile([PT, SQC], f32, tag="ps", name=f"qps{c}_{mt}")
                for k in range(KT):
                    nc.tensor.matmul(
                        ps[:], wq_sb[:, k, mt * PT:(mt + 1) * PT],
                        halves[k // 8][:, k % 8, :],
                        start=(k == 0), stop=(k == KT - 1))
                nc.scalar.copy(qc[:, mt, :], ps[:])
            for dst, wsb, nm in ((kc, wk_sb, "k"), (vtc, wv_sb, "v")):
                ps = mw.tile([PT, SQC], f32, tag="ps", name=f"ps{nm}{c}")
                for k in range(KT):
                    nc.tensor.matmul(
                        ps[:], wsb[:, k, :],
                        halves[k // 8][:, k % 8, :],
                        start=(k == 0), stop=(k == KT - 1))
                nc.scalar.copy(dst[:], ps[:])

            if c + 1 < NCHUNK:
                load_x(c + 1)

            # ---- rope(c) ------------------------------------------------
            for j in range(2):
                rope_pair(qc[:, j, :], qc[:, 2 + j, :],
                          cos_sb[:, csl], sin_sb[:, csl], f"q{c}_{j}")
            # k pair: rows 0:64 / 64:128 — stage B rows to base 0 via DMA
            bst = rtmp.tile([64, SQC], bf16, tag="t1", name=f"bst{c}")
            nc.sync.dma_start(out=bst[:], in_=kc[64:128, :])
            kt1 = rtmp.tile([64, SQC], bf16, tag="t2", name=f"kt1{c}")
            kt2 = rtmp.tile([64, SQC], bf16, tag="t3", name=f"kt2{c}")
            kt3 = rtmp.tile([64, SQC], bf16, tag="t1b", name=f"kt3{c}")
            kt4 = rtmp.tile([64, SQC], bf16, tag="t2b", name=f"kt4{c}")
            nc.vector.tensor_mul(kt1[:], kc[0:64, :], cos_sb[0:64, csl])
            nc.vector.tensor_mul(kt2[:], kc[0:64, :], sin_sb[0:64, csl])
            nc.vector.tensor_mul(kt3[:], bst[:], sin_sb[0:64, csl])
            nc.vector.tensor_mul(kt4[:], bst[:], cos_sb[0:64, csl])
            nc.vector.tensor_sub(kc[0:64, :], kt1[:], kt3[:])
            kbr = rtmp.tile([64, SQC], bf16, tag="t3b", name=f"kbr{c}")
            nc.vector.tensor_add(kbr[:], kt2[:], kt4[:])
            nc.sync.dma_start(out=kc[64:128, :], in_=kbr[:])

            # ---- kTrep(c): each 32-row group replicated x4 --------------
            for r in range(4):
                for slot in range(4):
                    nc.sync.dma_start(
                        out=kTrep[slot * 32:(slot + 1) * 32, r, csl],
                        in_=kc[r * 32:(r + 1) * 32, :])

            # ---- v(c): transpose vT chunk into v_sb ---------------------
            for tl in range(TPC):
                t = c * TPC + tl
                tp = mw.tile([PT, SQC], f32, tag="ps", name=f"tp{c}_{tl}")
                tpb = tp[:, 0:PT].bitcast(bf16)[:, 0:PT]
                nc.tensor.transpose(tpb,
                                    vtc[:, tl * PT:(tl + 1) * PT],
                                    id_sb[:])
                nc.vector.tensor_copy(v_sb[:, 0, t, 0:HD], tpb[:, 0:HD])
                nc.vector.tensor_copy(v_sb[:, 1, t, 0:HD], tpb[:, HD:2 * HD])

            # ---- attention(c) -------------------------------------------
            outc = otp.tile([PT, 4, SQC], f32, tag="outc", name=f"outc{c}")
            ntk = 4 * c + 4
            LAG = 2
            for g in range(2):
                for m in range(4):
                    msl = slice(m * 32, (m + 1) * 32)
                    av = aps.tile([PT, SQC], f32, tag="av",
                                  name=f"av{c}_{g}_{m}")
                    pbq = []
                    for tt in range(ntk + LAG):
                        if tt < ntk:
                            t = tt
                            ksl = slice(t * PT, (t + 1) * PT)
                            sc = sps.tile([PT, SQC], f32, tag="sc",
                                          name=f"sc{c}_{g}_{m}_{t}")
                            diag = t >= 4 * c
                            nc.tensor.matmul(
                                sc[:], kTrep[msl, g, ksl], qc[msl, g, :],
                                start=True, stop=False,
                                tile_position=(m * 32, 0))
                            nc.tensor.matmul(
                                sc[:], kTrep[msl, 2 + g, ksl],
                                qc[msl, 2 + g, :],
                                start=False, stop=not diag,
                                tile_position=(m * 32, 0))
                            if diag:
                                r = t - 4 * c
                                nc.tensor.matmul(
                                    sc[:], le_sb[:],
                                    dm_sb[:, 384 - 128 * r:MASK_W - 128 * r],
                                    start=False, stop=True)
                            pb = probs.tile([PT, SQC], bf16, tag="pb",
                                            name=f"pb{c}_{g}_{m}_{t}")
                            nc.scalar.activation(pb[:], sc[:], Exp)
                            pbq.append(pb)
                        if tt >= LAG:
                            t = tt - LAG
                            nc.tensor.matmul(
                                av[0:HD + 1, :], v_sb[:, g, t, :],
                                pbq[t][:],
                                start=(t == 0), stop=(t == ntk - 1))
                    qh = g * 4 + m
                    rc = rcp.tile([1, SQC], f32, tag="rc",
                                  name=f"rc{c}_{g}_{m}")
                    nc.vector.reciprocal(rc[:], av[HD:HD + 1, :])
                    bc = bcp.tile([64, SQC], f32, tag="bc",
                                  name=f"bc{c}_{g}_{m}")
                    nc.gpsimd.partition_broadcast(bc[:], rc[:])
                    dst = outc[(qh % 2) * HD:(qh % 2 + 1) * HD, qh // 2, :]
                    if qh % 2 == 0:
                        nc.vector.tensor_mul(mc(dst), av[0:HD, :], bc[:])
                    else:
                        st = osg.tile([64, SQC], f32, tag="st",
                                      name=f"st{c}_{g}_{m}")
                        nc.vector.tensor_mul(st[:], av[0:HD, :], bc[:])
                        nc.sync.dma_start(out=mc(dst), in_=mc(st[:]))

            # ---- wo(c) --------------------------------------------------
            for tl in range(TPC):
                tt = c * TPC + tl
                yt = ysb.tile([PT, D], f32, tag="yt", name=f"yt{c}_{tl}")
                for nk in range(4):
                    yp = mw.tile([PT, SQC], f32, tag="ps",
                                 name=f"yp{c}_{tl}_{nk}")
                    for k4 in range(4):
                        nc.tensor.matmul(
                            yp[:], mc(outc[:, k4, tl * PT:(tl + 1) * PT]),
                            mc(wo_sb[:, k4, nk * SQC:(nk + 1) * SQC]),
                            start=(k4 == 0), stop=(k4 == 3))
                    nc.vector.tensor_copy(yt[:, nk * SQC:(nk + 1) * SQC],
                                          yp[:])
                nc.sync.dma_start(out=y_part[tt * PT:(tt + 1) * PT, :],
                                  in_=yt[:])

            # ---- RS(c): reduce-scatter this chunk (last chunk split per
            #      tok-tile so the kernel tail is one small collective) ----
            if c < NCHUNK - 1:
                nc.gpsimd.collective_compute(
                    "ReduceScatter", mybir.AluOpType.add,
                    replica_groups=groups,
                    ins=[y_part.ap()[csl, :]],
                    outs=[y_rs.ap()[c * PT:(c + 1) * PT, :]])
                nc.sync.dma_start(
                    out=y_out.ap()[c * PT:(c + 1) * PT, :],
                    in_=y_rs.ap()[c * PT:(c + 1) * PT, :])
            else:
                q = PT // TPC   # 32 rows out per tok-tile RS
                for tl in range(TPC):
                    tt = c * TPC + tl
                    nc.gpsimd.collective_compute(
                        "ReduceScatter", mybir.AluOpType.add,
                        replica_groups=groups,
                        ins=[y_part.ap()[tt * PT:(tt + 1) * PT, :]],
                        outs=[y_rs.ap()[c * PT + tl * q:
                                        c * PT + (tl + 1) * q, :]])
                    nc.sync.dma_start(
                        out=y_out.ap()[c * PT + tl * q:
                                       c * PT + (tl + 1) * q, :],
                        in_=y_rs.ap()[c * PT + tl * q:
                                      c * PT + (tl + 1) * q, :])

    nc.compile()
    return nc


@functools.lru_cache(maxsize=2)
def _get_program(mm_dtype_name="float32r"):
    return _build_program(mm_dtype_name)


def _host_inputs(x, wq, wk, wv, wo, cos, sin):
    """Build the 8 per-core input maps."""
    import ml_dtypes

    perm_q = np.empty(HPC * HD, np.int64)
    for rho in range(HPC * HD):
        blk, rem = divmod(rho, HPC * HD // 2)
        h, i = divmod(rem, 32)
        perm_q[rho] = h * HD + 2 * i + blk
    perm_k = np.empty(KVPC * HD, np.int64)
    for rho in range(KVPC * HD):
        blk, rem = divmod(rho, KVPC * HD // 2)
        kv, i = divmod(rem, 32)
        perm_k[rho] = kv * HD + 2 * i + blk

    reps = np.tile(np.arange(32), 4)
    cosr = np.ascontiguousarray(cos.T[reps]).astype(ml_dtypes.bfloat16)
    sinr = np.ascontiguousarray(sin.T[reps]).astype(ml_dtypes.bfloat16)

    xts = []
    for b in range(B):
        xt = x[b].T.reshape(D, NCHUNK, SQC)       # [D, 4, 512]
        xts.append(np.ascontiguousarray(xt.transpose(1, 0, 2))
                   .astype(ml_dtypes.bfloat16))

    scale = np.float32(1.0 / np.sqrt(HD))
    in_maps = []
    for core in range(N_CORES):
        b, hg = divmod(core, GROUP)
        qcols = slice(hg * HPC * HD, (hg + 1) * HPC * HD)
        kcols = slice(hg * KVPC * HD, (hg + 1) * KVPC * HD)
        wq_c = (wq[:, qcols] * scale)[:, perm_q].astype(ml_dtypes.bfloat16)
        wk_c = wk[:, kcols][:, perm_k].astype(ml_dtypes.bfloat16)
        wv_c = np.ascontiguousarray(wv[:, kcols]).astype(ml_dtypes.bfloat16)
        wo_c = np.ascontiguousarray(wo[qcols, :])
        in_maps.append({
            "xt": xts[b],
            "wq": np.ascontiguousarray(wq_c),
            "wk": np.ascontiguousarray(wk_c),
            "wv": wv_c,
            "wo": wo_c,
            "cosr": cosr,
            "sinr": sinr,
        })
    return in_maps


def _assemble(results):
    """results[core]["y"]: chunks 0..2 are [128]-row RS quarters; chunk 3
    was reduce-scattered per tok-tile ([32]-row quarters)."""
    out = np.empty((B, S, D), np.float32)
    q = PT // TPC
    for b in range(B):
        for r in range(GROUP):
            y = results[b * GROUP + r]["y"]
            for c in range(NCHUNK - 1):
                rows = slice(c * SQC + r * PT, c * SQC + (r + 1) * PT)
                out[b, rows, :] = y[c * PT:(c + 1) * PT, :]
            c = NCHUNK - 1
            for tl in range(TPC):
                tt = c * TPC + tl
                rows = slice(tt * PT + r * q, tt * PT + (r + 1) * q)
                out[b, rows, :] = y[c * PT + tl * q:c * PT + (tl + 1) * q, :]
    return out


def _is_causal(mask):
    if mask.shape != (S, S):
        return False
    expect = np.where(np.tril(np.ones((S, S), bool)), np.float32(0.0),
                      np.float32(NEG))
    return np.array_equal(mask, expect)


def _numpy_fallback(x, wq, wk, wv, wo, cos, sin, mask):
    """Exact reference math on host (only used if mask isn't causal)."""
    xq = (x @ wq).reshape(B, S, H, HD)
    xk = (x @ wk).reshape(B, S, KVH, HD)
    xv = (x @ wv).reshape(B, S, KVH, HD)

    def rope(t):
        tr = t.reshape(*t.shape[:-1], HD // 2, 2)
        a, b = tr[..., 0], tr[..., 1]
        c = cos[None, :, None, :]
        s_ = sin[None, :, None, :]
        out = np.stack([a * c - b * s_, a * s_ + b * c], axis=-1)
        return out.reshape(t.shape)

    xq, xk = rope(xq), rope(xk)
    xk = np.repeat(xk, H // KVH, axis=2)
    xv = np.repeat(xv, H // KVH, axis=2)
    q = xq.transpose(0, 2, 1, 3)
    k = xk.transpose(0, 2, 1, 3)
    v = xv.transpose(0, 2, 1, 3)
    sc = np.einsum("bhqd,bhkd->bhqk", q, k) / np.sqrt(np.float32(HD))
    sc = sc + mask[None, None]
    sc = sc - sc.max(-1, keepdims=True)
    p = np.exp(sc)
    p /= p.sum(-1, keepdims=True)
    out = np.einsum("bhqk,bhkd->bhqd", p, v)
    out = out.transpose(0, 2, 1, 3).reshape(B, S, H * HD)
    return (out @ wo).astype(np.float32)


def _ensure_ntff_hook():
    """Provide antenv.axon_hooks (missing on this image) so trace=True works."""
    try:
        from antenv.axon_hooks import get_axon_ntff_profile_hook  # noqa: F401
        return True
    except ImportError:
        pass
    try:
        import types
        import antenv
        from trn_agent_boot.trn_boot import _ntff_profile_via_ctypes

        mod = types.ModuleType("antenv.axon_hooks")
        _state = {"hook": None}
        mod.set_axon_ntff_profile_hook = \
            lambda h: _state.__setitem__("hook", h)
        mod.get_axon_ntff_profile_hook = lambda: _state["hook"]
        sys.modules["antenv.axon_hooks"] = mod
        antenv.axon_hooks = mod
        mod.set_axon_ntff_profile_hook(
            _ntff_profile_via_ctypes("/opt/axon/libaxon_pjrt.so"))
        return mod.get_axon_ntff_profile_hook() is not None
    except Exception:
        return False


def kernel(x, wq, wk, wv, wo, cos, sin, mask):
    x = np.asarray(x, np.float32)
    wq = np.asarray(wq, np.float32)
    wk = np.asarray(wk, np.float32)
    wv = np.asarray(wv, np.float32)
    wo = np.asarray(wo, np.float32)
    cos = np.asarray(cos, np.float32)
    sin = np.asarray(sin, np.float32)
    mask = np.asarray(mask, np.float32)

    if not _is_causal(mask):
        return _numpy_fallback(x, wq, wk, wv, wo, cos, sin, mask)

    from concourse.bass_utils import run_bass_kernel_spmd

    nc = _get_program(os.environ.get("ATTN_MM_DTYPE", "float32r"))
    in_maps = _host_inputs(x, wq, wk, wv, wo, cos, sin)
    trace = bool(int(os.environ.get("ATTN_TRACE", "0")))
    if trace and not _ensure_ntff_hook():
        trace = False
    res = run_bass_kernel_spmd(nc, in_maps, core_ids=list(range(N_CORES)),
                               trace=trace)
    if trace:
        kernel.last_exec_time_ns = res.exec_time_ns
        kernel.last_results = res
    return _assemble(res.results)


# revision 22
# speedup vs baseline: 1.1406x; 1.0756x over previous
"""Trainium2 Bass kernel for GQA attention (B=2, S=2048, D=2048, H=32, KVH=8).

Sharding: batch data-parallel across 2 groups of 4 cores; within a group,
4-way tensor parallel over heads (8 q heads + their 2 kv heads per core).
Device-side per-chunk ReduceScatter(add) over each 4-core group after the wo
matmul; the host concatenates the token slices.

The device program is identical on all 8 cores (SPMD); all per-core
variation (batch slice, head slice) is carried by the input data.

v2 structure: a software pipeline over 512-token chunks —
   proj(c) -> rope(c) -> kTrep/v(c) -> attention(c) -> wo(c) -> RS(c)
so the tensor engine stays dense (HAM stays warm) and the collective
overlaps compute. Projections and the wo matmul run in fp32r; the
attention path (q/k/v/probs/cos/mask) runs in bf16 (fp32 PSUM accum).

Layout notes:
 - Host passes x pre-transposed (xT, chunk-major); every matmul consumes xT
   directly (no on-device transposes of activations).
 - wq/wk columns are permuted on host into an "even dims block / odd dims
   block" (A/B) layout so RoPE is full-partition DVE work; wq carries the
   1/sqrt(HD) scale (exact power of two).
 - Scores are computed transposed (scoresT[sk, sq]) so probsT feeds the AV
   matmul directly with no transposes in the attention path.
 - Causal mask: one extra accumulating matmul per diagonal tile,
   LEones[k,p]=[k<=p] x shifted -1e9 diagonal, adds -1e9 to masked entries.
 - Softmax denominators ride along as a ones column in v (M=65 AV matmul);
   normalization multiplies by the partition-broadcast reciprocal.
"""

import os
import sys
import functools

import numpy as np

if "/opt/trn_rl_repo" not in sys.path:
    sys.path.insert(0, "/opt/trn_rl_repo")

B, S, D = 2, 2048, 2048
H, KVH = 32, 8
HD = D // H            # 64
N_CORES = 8
GROUP = 4              # cores per batch group (tensor parallel width)
HPC = 8                # query heads per core
KVPC = 2               # kv heads per core
SQC = 512              # sq chunk (psum bank width in fp32)
PT = 128               # partition tile
KT = D // PT           # 16 contraction tiles
NT = S // PT           # 16 token tiles
NCHUNK = S // SQC      # 4
TPC = SQC // PT        # tok tiles per chunk (4)
MASK_W = 896           # width of the shifted-diagonal mask table
NEG = -1e9


def _build_program(mm_dtype_name="float32r"):
    import concourse.bass as bass
    import concourse.bacc as bacc
    import concourse.mybir as mybir
    import concourse.tile as tile
    import ml_dtypes
    from contextlib import ExitStack

    f32 = mybir.dt.float32
    bf16 = mybir.dt.bfloat16
    mmdt = getattr(mybir.dt, mm_dtype_name)

    def mc(ap):  # bitcast for fp32r matmul operands/producers
        return ap.bitcast(mmdt) if mmdt != f32 else ap

    nc = bacc.Bacc("TRN2", target_bir_lowering=False, debug=False,
                   num_devices=N_CORES)

    # ---- dram parameters -------------------------------------------------
    xT_d = nc.dram_tensor("xt", [NCHUNK, D, SQC], bf16, kind="ExternalInput")
    wq_d = nc.dram_tensor("wq", [D, HPC * HD], bf16, kind="ExternalInput")
    wk_d = nc.dram_tensor("wk", [D, KVPC * HD], bf16, kind="ExternalInput")
    wv_d = nc.dram_tensor("wv", [D, KVPC * HD], bf16, kind="ExternalInput")
    wo_d = nc.dram_tensor("wo", [HPC * HD, D], f32, kind="ExternalInput")
    cos_d = nc.dram_tensor("cosr", [PT, S], bf16, kind="ExternalInput")
    sin_d = nc.dram_tensor("sinr", [PT, S], bf16, kind="ExternalInput")
    y_out = nc.dram_tensor("y", [S // GROUP, D], f32, kind="ExternalOutput")

    y_part = nc.dram_tensor("y_part", [S, D], f32)
    y_rs = nc.dram_tensor("y_rs", [S // GROUP, D], f32)

    # ---- inline constants ------------------------------------------------
    leones = np.zeros((PT, PT), np.float32)      # leones[k, p] = 1 if k <= p
    for k in range(PT):
        leones[k, k:] = 1.0
    dmaster = np.zeros((PT, MASK_W), np.float32)
    # col m: -1e9 one-hot at k = m-383 for m in [384, 510]; row 0 for m<384
    dmaster[0, :384] = NEG
    for m in range(384, 511):
        dmaster[m - 383, m] = NEG
    ident = np.eye(PT, dtype=ml_dtypes.bfloat16)
    ones1 = np.ones((PT, 1), ml_dtypes.bfloat16)

    le_d = nc.inline_tensor(leones.astype(ml_dtypes.bfloat16), "leones")
    dm_d = nc.inline_tensor(dmaster.astype(ml_dtypes.bfloat16), "dmaster")
    id_d = nc.inline_tensor(ident, "ident")
    on_d = nc.inline_tensor(ones1, "ones1")

    Exp = mybir.ActivationFunctionType.Exp
    groups = [[0, 1, 2, 3], [4, 5, 6, 7]]

    with tile.TileContext(nc) as tc, ExitStack() as ctx:
        keep = ctx.enter_context(tc.tile_pool(name="keep", bufs=1))
        # packed K cache: krp[kv] rows = [kv(a32 b32); kv(a32 b32)] replicated
        krp0 = keep.tile([PT, S], bf16)
        krp1 = keep.tile([PT, S], bf16)
        krp = [krp0, krp1]
        v_sb = keep.tile([PT, KVPC, NT, HD + 1], bf16)   # col 64 = ones
        cos_sb = keep.tile([PT, S], bf16)
        sin_sb = keep.tile([PT, S], bf16)
        le_sb = keep.tile([PT, PT], bf16)
        dm_sb = keep.tile([PT, MASK_W], bf16)
        id_sb = keep.tile([PT, PT], bf16)
        wq_sb = keep.tile([PT, KT, HPC * HD], bf16)
        wk_sb = keep.tile([PT, KT, KVPC * HD], bf16)
        wv_sb = keep.tile([PT, KT, KVPC * HD], bf16)
        wo_sb = keep.tile([PT, 4, D], f32)

        xcache = {}

        def load_x(c):
            halves = []
            for hf in range(2):
                xt = xpool.tile([PT, KT // 2, SQC], bf16, tag="xt",
                                name=f"xt{c}_{hf}")
                nc.sync.dma_start(
                    out=xt[:],
                    in_=xT_d[c].rearrange("(k p) n -> p k n", p=PT)
                    [:, hf * (KT // 2):(hf + 1) * (KT // 2), :])
                halves.append(xt)
            xcache[c] = halves

        nc.sync.dma_start(out=wq_sb[:],
                          in_=wq_d.ap().rearrange("(k p) n -> p k n", p=PT))
        nc.sync.dma_start(out=wk_sb[:],
                          in_=wk_d.ap().rearrange("(k p) n -> p k n", p=PT))
        nc.sync.dma_start(out=wv_sb[:],
                          in_=wv_d.ap().rearrange("(k p) n -> p k n", p=PT))
        nc.sync.dma_start(out=cos_sb[:], in_=cos_d[:])
        nc.sync.dma_start(out=sin_sb[:], in_=sin_d[:])
        nc.sync.dma_start(out=le_sb[:], in_=le_d[:])
        nc.sync.dma_start(out=dm_sb[:], in_=dm_d[:])
        nc.sync.dma_start(out=id_sb[:], in_=id_d[:])
        nc.sync.dma_start(out=mc(wo_sb[:]),
                          in_=mc(wo_d.ap().rearrange("(k p) n -> p k n", p=PT)))
        # ones column of v (every (kv, t) slot)
        ones_src = bass.AP(tensor=on_d.ap().tensor, offset=0,
                           ap=[[1, PT], [0, KVPC * NT], [1, 1]])
        vcol = v_sb[:, :, :, HD:HD + 1]
        ones_dst = bass.AP(tensor=vcol.tensor, offset=vcol.offset,
                           ap=[list(vcol.ap[0]), [HD + 1, KVPC * NT], [1, 1]])
        nc.sync.dma_start(out=ones_dst, in_=ones_src)

        xpool = ctx.enter_context(tc.tile_pool(name="xp", bufs=4))
        qpool = ctx.enter_context(tc.tile_pool(name="qp", bufs=2))
        qppool = ctx.enter_context(tc.tile_pool(name="qpp", bufs=2))
        kpool = ctx.enter_context(tc.tile_pool(name="kp", bufs=2))
        vtp = ctx.enter_context(tc.tile_pool(name="vtp", bufs=2))
        otp = ctx.enter_context(tc.tile_pool(name="otp", bufs=2))
        rtmp = ctx.enter_context(tc.tile_pool(name="rtmp", bufs=1))
        probs = ctx.enter_context(tc.tile_pool(name="probs", bufs=8))
        bcp = ctx.enter_context(tc.tile_pool(name="bcp", bufs=2))
        rcp = ctx.enter_context(tc.tile_pool(name="rcp", bufs=2))
        osg = ctx.enter_context(tc.tile_pool(name="osg", bufs=2))
        ysb = ctx.enter_context(tc.tile_pool(name="ysb", bufs=3))
        mw = ctx.enter_context(tc.tile_pool(name="mw", bufs=2, space="PSUM"))
        sps = ctx.enter_context(tc.tile_pool(name="sps", bufs=4, space="PSUM"))
        aps = ctx.enter_context(tc.tile_pool(name="aps", bufs=2, space="PSUM"))

        load_x(0)

        def rope_pair(a, b, cs, sn, nm):
            """a' = a*cos - b*sin ; b' = a*sin + b*cos (bf16, in place)."""
            t1 = rtmp.tile(a.shape, bf16, tag="t1", name=f"t1{nm}")
            t2 = rtmp.tile(a.shape, bf16, tag="t2", name=f"t2{nm}")
            t3 = rtmp.tile(a.shape, bf16, tag="t3", name=f"t3{nm}")
            nc.vector.tensor_mul(t1[:], a, cs)
            nc.vector.tensor_mul(t2[:], a, sn)
            nc.vector.tensor_mul(t3[:], b, sn)
            nc.vector.tensor_sub(a, t1[:], t3[:])
            t4 = rtmp.tile(a.shape, bf16, tag="t3", name=f"t4{nm}")
            nc.vector.tensor_mul(t4[:], b, cs)
            nc.vector.tensor_add(b, t2[:], t4[:])

        for c in range(NCHUNK):
            csl = slice(c * SQC, (c + 1) * SQC)

            # ---- proj(c): qT chunk (bf16), kT chunk (bf16), vT chunk ----
            if c not in xcache:
                load_x(c)
            halves = xcache.pop(c)

            qc = qpool.tile([PT, 4, SQC], bf16, tag="qc", name=f"qc{c}")
            kc = kpool.tile([PT, SQC], bf16, tag="kc", name=f"kc{c}")
            vtc = vtp.tile([PT, SQC], bf16, tag="vtc", name=f"vtc{c}")
            for mt in range(4):
                ps = mw.tile([PT, SQC], f32, tag="ps", name=f"qps{c}_{mt}")
                for k in range(KT):
                    nc.tensor.matmul(
                        ps[:], wq_sb[:, k, mt * PT:(mt + 1) * PT],
                        halves[k // 8][:, k % 8, :],
                        start=(k == 0), stop=(k == KT - 1))
                nc.scalar.copy(qc[:, mt, :], ps[:])
            for dst, wsb, nm in ((kc, wk_sb, "k"), (vtc, wv_sb, "v")):
                ps = mw.tile([PT, SQC], f32, tag="ps", name=f"ps{nm}{c}")
                for k in range(KT):
                    nc.tensor.matmul(
                        ps[:], wsb[:, k, :],
                        halves[k // 8][:, k % 8, :],
                        start=(k == 0), stop=(k == KT - 1))
                nc.scalar.copy(dst[:], ps[:])

            if c + 1 < NCHUNK:
                load_x(c + 1)

            # ---- rope(c) ------------------------------------------------
            for j in range(2):
                rope_pair(qc[:, j, :], qc[:, 2 + j, :],
                          cos_sb[:, csl], sin_sb[:, csl], f"q{c}_{j}")
            # k pair: rows 0:64 / 64:128 — stage B rows to base 0 via DMA
            bst = rtmp.tile([64, SQC], bf16, tag="t1", name=f"bst{c}")
            nc.sync.dma_start(out=bst[:], in_=kc[64:128, :])
            kt1 = rtmp.tile([64, SQC], bf16, tag="t2", name=f"kt1{c}")
            kt2 = rtmp.tile([64, SQC], bf16, tag="t3", name=f"kt2{c}")
            kt3 = rtmp.tile([64, SQC], bf16, tag="t1b", name=f"kt3{c}")
            kt4 = rtmp.tile([64, SQC], bf16, tag="t2b", name=f"kt4{c}")
            nc.vector.tensor_mul(kt1[:], kc[0:64, :], cos_sb[0:64, csl])
            nc.vector.tensor_mul(kt2[:], kc[0:64, :], sin_sb[0:64, csl])
            nc.vector.tensor_mul(kt3[:], bst[:], sin_sb[0:64, csl])
            nc.vector.tensor_mul(kt4[:], bst[:], cos_sb[0:64, csl])
            nc.vector.tensor_sub(kc[0:64, :], kt1[:], kt3[:])
            kbr = rtmp.tile([64, SQC], bf16, tag="t3b", name=f"kbr{c}")
            nc.vector.tensor_add(kbr[:], kt2[:], kt4[:])
            nc.sync.dma_start(out=kc[64:128, :], in_=kbr[:])

            # ---- pack(c): head-contiguous q (K=64 scores) ---------------
            # qcp[j] rows: head 2j at [0:64] (a32 b32), head 2j+1 at [64:128]
            qcp = qppool.tile([PT, 4, SQC], bf16, tag="qcp", name=f"qcp{c}")
            for qh in range(HPC):
                so = slice((qh % 4) * 32, (qh % 4) * 32 + 32)
                do = (qh % 2) * 64
                nc.sync.dma_start(out=qcp[do:do + 32, qh // 2, :],
                                  in_=qc[so, qh // 4, :])
                nc.sync.dma_start(out=qcp[do + 32:do + 64, qh // 2, :],
                                  in_=qc[so, 2 + qh // 4, :])
            # krp[kv] rows [0:64]=[64:128] = kv's (a32 b32)
            for kv in range(KVPC):
                for rep in range(2):
                    ro = rep * 64
                    nc.sync.dma_start(
                        out=krp[kv][ro:ro + 32, csl],
                        in_=kc[kv * 32:(kv + 1) * 32, :])
                    nc.sync.dma_start(
                        out=krp[kv][ro + 32:ro + 64, csl],
                        in_=kc[64 + kv * 32:64 + (kv + 1) * 32, :])

            # ---- v(c): transpose vT chunk into v_sb ---------------------
            for tl in range(TPC):
                t = c * TPC + tl
                tp = mw.tile([PT, SQC], f32, tag="ps", name=f"tp{c}_{tl}")
                tpb = tp[:, 0:PT].bitcast(bf16)[:, 0:PT]
                nc.tensor.transpose(tpb,
                                    vtc[:, tl * PT:(tl + 1) * PT],
                                    id_sb[:])
                nc.vector.tensor_copy(v_sb[:, 0, t, 0:HD], tpb[:, 0:HD])
                nc.vector.tensor_copy(v_sb[:, 1, t, 0:HD], tpb[:, HD:2 * HD])

            # ---- attention(c) -------------------------------------------
            outc = otp.tile([PT, 4, SQC], f32, tag="outc", name=f"outc{c}")
            ntk = 4 * c + 4
            LAG = 2
            for qh in range(HPC):
                    g = qh // 4            # kv group
                    base = (qh % 2) * 64
                    av = aps.tile([PT, SQC], f32, tag="av",
                                  name=f"av{c}_{qh}")
                    pbq = []
                    for tt in range(ntk + LAG):
                        if tt < ntk:
                            t = tt
                            ksl = slice(t * PT, (t + 1) * PT)
                            sc = sps.tile([PT, SQC], f32, tag="sc",
                                          name=f"sc{c}_{qh}_{t}")
                            diag = t >= 4 * c
                            nc.tensor.matmul(
                                sc[:], krp[g][base:base + 64, ksl],
                                qcp[base:base + 64, qh // 2, :],
                                start=True, stop=not diag,
                                tile_position=(base, 0))
                            if diag:
                                r = t - 4 * c
                                nc.tensor.matmul(
                                    sc[:], le_sb[:],
                                    dm_sb[:, 384 - 128 * r:MASK_W - 128 * r],
                                    start=False, stop=True)
                            pb = probs.tile([PT, SQC], bf16, tag="pb",
                                            name=f"pb{c}_{qh}_{t}")
                            nc.scalar.activation(pb[:], sc[:], Exp)
                            pbq.append(pb)
                        if tt >= LAG:
                            t = tt - LAG
                            nc.tensor.matmul(
                                av[0:HD + 1, :], v_sb[:, g, t, :],
                                pbq[t][:],
                                start=(t == 0), stop=(t == ntk - 1))
                    rc = rcp.tile([1, SQC], f32, tag="rc",
                                  name=f"rc{c}_{qh}")
                    nc.vector.reciprocal(rc[:], av[HD:HD + 1, :])
                    bc = bcp.tile([64, SQC], f32, tag="bc",
                                  name=f"bc{c}_{qh}")
                    nc.gpsimd.partition_broadcast(bc[:], rc[:])
                    dst = outc[(qh % 2) * HD:(qh % 2 + 1) * HD, qh // 2, :]
                    if qh % 2 == 0:
                        nc.vector.tensor_mul(mc(dst), av[0:HD, :], bc[:])
                    else:
                        st = osg.tile([64, SQC], f32, tag="st",
                                      name=f"st{c}_{qh}")
                        nc.vector.tensor_mul(st[:], av[0:HD, :], bc[:])
                        nc.sync.dma_start(out=mc(dst), in_=mc(st[:]))

            # ---- wo(c) --------------------------------------------------
            for tl in range(TPC):
                tt = c * TPC + tl
                yt = ysb.tile([PT, D], f32, tag="yt", name=f"yt{c}_{tl}")
                for nk in range(4):
                    yp = mw.tile([PT, SQC], f32, tag="ps",
                                 name=f"yp{c}_{tl}_{nk}")
                    for k4 in range(4):
                        nc.tensor.matmul(
                            yp[:], mc(outc[:, k4, tl * PT:(tl + 1) * PT]),
                            mc(wo_sb[:, k4, nk * SQC:(nk + 1) * SQC]),
                            start=(k4 == 0), stop=(k4 == 3))
                    nc.vector.tensor_copy(yt[:, nk * SQC:(nk + 1) * SQC],
                                          yp[:])
                nc.sync.dma_start(out=y_part[tt * PT:(tt + 1) * PT, :],
                                  in_=yt[:])

            # ---- RS(c): reduce-scatter this chunk (last chunk split per
            #      tok-tile so the kernel tail is one small collective) ----
            if c < NCHUNK - 1:
                nc.gpsimd.collective_compute(
                    "ReduceScatter", mybir.AluOpType.add,
                    replica_groups=groups,
                    ins=[y_part.ap()[csl, :]],
                    outs=[y_rs.ap()[c * PT:(c + 1) * PT, :]])
                nc.sync.dma_start(
                    out=y_out.ap()[c * PT:(c + 1) * PT, :],
                    in_=y_rs.ap()[c * PT:(c + 1) * PT, :])
            else:
                q = PT // TPC   # 32 rows out per tok-tile RS
                for tl in range(TPC):
                    tt = c * TPC + tl
                    nc.gpsimd.collective_compute(
                        "ReduceScatter", mybir.AluOpType.add,
                        replica_groups=groups,
                        ins=[y_part.ap()[tt * PT:(tt + 1) * PT, :]],
                        outs=[y_rs.ap()[c * PT + tl * q:
                                        c * PT + (tl + 1) * q, :]])
                    nc.sync.dma_start(
                        out=y_out.ap()[c * PT + tl * q:
                                       c * PT + (tl + 1) * q, :],
                        in_=y_rs.ap()[c * PT + tl * q:
                                      c * PT + (tl + 1) * q, :])

    nc.compile()
    return nc


@functools.lru_cache(maxsize=2)
def _get_program(mm_dtype_name="float32r"):
    return _build_program(mm_dtype_name)


def _host_inputs(x, wq, wk, wv, wo, cos, sin):
    """Build the 8 per-core input maps."""
    import ml_dtypes

    perm_q = np.empty(HPC * HD, np.int64)
    for rho in range(HPC * HD):
        blk, rem = divmod(rho, HPC * HD // 2)
        h, i = divmod(rem, 32)
        perm_q[rho] = h * HD + 2 * i + blk
    perm_k = np.empty(KVPC * HD, np.int64)
    for rho in range(KVPC * HD):
        blk, rem = divmod(rho, KVPC * HD // 2)
        kv, i = divmod(rem, 32)
        perm_k[rho] = kv * HD + 2 * i + blk

    reps = np.tile(np.arange(32), 4)
    cosr = np.ascontiguousarray(cos.T[reps]).astype(ml_dtypes.bfloat16)
    sinr = np.ascontiguousarray(sin.T[reps]).astype(ml_dtypes.bfloat16)

    xts = []
    for b in range(B):
        xt = x[b].T.reshape(D, NCHUNK, SQC)       # [D, 4, 512]
        xts.append(np.ascontiguousarray(xt.transpose(1, 0, 2))
                   .astype(ml_dtypes.bfloat16))

    scale = np.float32(1.0 / np.sqrt(HD))
    in_maps = []
    for core in range(N_CORES):
        b, hg = divmod(core, GROUP)
        qcols = slice(hg * HPC * HD, (hg + 1) * HPC * HD)
        kcols = slice(hg * KVPC * HD, (hg + 1) * KVPC * HD)
        wq_c = (wq[:, qcols] * scale)[:, perm_q].astype(ml_dtypes.bfloat16)
        wk_c = wk[:, kcols][:, perm_k].astype(ml_dtypes.bfloat16)
        wv_c = np.ascontiguousarray(wv[:, kcols]).astype(ml_dtypes.bfloat16)
        wo_c = np.ascontiguousarray(wo[qcols, :])
        in_maps.append({
            "xt": xts[b],
            "wq": np.ascontiguousarray(wq_c),
            "wk": np.ascontiguousarray(wk_c),
            "wv": wv_c,
            "wo": wo_c,
            "cosr": cosr,
            "sinr": sinr,
        })
    return in_maps


def _assemble(results):
    """results[core]["y"]: chunks 0..2 are [128]-row RS quarters; chunk 3
    was reduce-scattered per tok-tile ([32]-row quarters)."""
    out = np.empty((B, S, D), np.float32)
    q = PT // TPC
    for b in range(B):
        for r in range(GROUP):
            y = results[b * GROUP + r]["y"]
            for c in range(NCHUNK - 1):
                rows = slice(c * SQC + r * PT, c * SQC + (r + 1) * PT)
                out[b, rows, :] = y[c * PT:(c + 1) * PT, :]
            c = NCHUNK - 1
            for tl in range(TPC):
                tt = c * TPC + tl
                rows = slice(tt * PT + r * q, tt * PT + (r + 1) * q)
                out[b, rows, :] = y[c * PT + tl * q:c * PT + (tl + 1) * q, :]
    return out


def _is_causal(mask):
    if mask.shape != (S, S):
        return False
    expect = np.where(np.tril(np.ones((S, S), bool)), np.float32(0.0),
                      np.float32(NEG))
    return np.array_equal(mask, expect)


def _numpy_fallback(x, wq, wk, wv, wo, cos, sin, mask):
    """Exact reference math on host (only used if mask isn't causal)."""
    xq = (x @ wq).reshape(B, S, H, HD)
    xk = (x @ wk).reshape(B, S, KVH, HD)
    xv = (x @ wv).reshape(B, S, KVH, HD)

    def rope(t):
        tr = t.reshape(*t.shape[:-1], HD // 2, 2)
        a, b = tr[..., 0], tr[..., 1]
        c = cos[None, :, None, :]
        s_ = sin[None, :, None, :]
        out = np.stack([a * c - b * s_, a * s_ + b * c], axis=-1)
        return out.reshape(t.shape)

    xq, xk = rope(xq), rope(xk)
    xk = np.repeat(xk, H // KVH, axis=2)
    xv = np.repeat(xv, H // KVH, axis=2)
    q = xq.transpose(0, 2, 1, 3)
    k = xk.transpose(0, 2, 1, 3)
    v = xv.transpose(0, 2, 1, 3)
    sc = np.einsum("bhqd,bhkd->bhqk", q, k) / np.sqrt(np.float32(HD))
    sc = sc + mask[None, None]
    sc = sc - sc.max(-1, keepdims=True)
    p = np.exp(sc)
    p /= p.sum(-1, keepdims=True)
    out = np.einsum("bhqk,bhkd->bhqd", p, v)
    out = out.transpose(0, 2, 1, 3).reshape(B, S, H * HD)
    return (out @ wo).astype(np.float32)


def _ensure_ntff_hook():
    """Provide antenv.axon_hooks (missing on this image) so trace=True works."""
    try:
        from antenv.axon_hooks import get_axon_ntff_profile_hook  # noqa: F401
        return True
    except ImportError:
        pass
    try:
        import types
        import antenv
        from trn_agent_boot.trn_boot import _ntff_profile_via_ctypes

        mod = types.ModuleType("antenv.axon_hooks")
        _state = {"hook": None}
        mod.set_axon_ntff_profile_hook = \
            lambda h: _state.__setitem__("hook", h)
        mod.get_axon_ntff_profile_hook = lambda: _state["hook"]
        sys.modules["antenv.axon_hooks"] = mod
        antenv.axon_hooks = mod
        mod.set_axon_ntff_profile_hook(
            _ntff_profile_via_ctypes("/opt/axon/libaxon_pjrt.so"))
        return mod.get_axon_ntff_profile_hook() is not None
    except Exception:
        return False


def kernel(x, wq, wk, wv, wo, cos, sin, mask):
    x = np.asarray(x, np.float32)
    wq = np.asarray(wq, np.float32)
    wk = np.asarray(wk, np.float32)
    wv = np.asarray(wv, np.float32)
    wo = np.asarray(wo, np.float32)
    cos = np.asarray(cos, np.float32)
    sin = np.asarray(sin, np.float32)
    mask = np.asarray(mask, np.float32)

    if not _is_causal(mask):
        return _numpy_fallback(x, wq, wk, wv, wo, cos, sin, mask)

    from concourse.bass_utils import run_bass_kernel_spmd

    nc = _get_program(os.environ.get("ATTN_MM_DTYPE", "float32r"))
    in_maps = _host_inputs(x, wq, wk, wv, wo, cos, sin)
    trace = bool(int(os.environ.get("ATTN_TRACE", "0")))
    if trace and not _ensure_ntff_hook():
        trace = False
    res = run_bass_kernel_spmd(nc, in_maps, core_ids=list(range(N_CORES)),
                               trace=trace)
    if trace:
        kernel.last_exec_time_ns = res.exec_time_ns
        kernel.last_results = res
    return _assemble(res.results)


# revision 24
# speedup vs baseline: 1.3414x; 1.1761x over previous
"""Trainium2 Bass kernel for GQA attention (B=2, S=2048, D=2048, H=32, KVH=8).

Sharding: batch data-parallel across 2 groups of 4 cores; within a group,
4-way tensor parallel over heads (8 q heads + their 2 kv heads per core).
Device-side per-chunk ReduceScatter(add) over each 4-core group after the wo
matmul; the host concatenates the token slices.

The device program is identical on all 8 cores (SPMD); all per-core
variation (batch slice, head slice) is carried by the input data.

v2 structure: a software pipeline over 512-token chunks —
   proj(c) -> rope(c) -> kTrep/v(c) -> attention(c) -> wo(c) -> RS(c)
so the tensor engine stays dense (HAM stays warm) and the collective
overlaps compute. Projections and the wo matmul run in fp32r; the
attention path (q/k/v/probs/cos/mask) runs in bf16 (fp32 PSUM accum).

Layout notes:
 - Host passes x pre-transposed (xT, chunk-major); every matmul consumes xT
   directly (no on-device transposes of activations).
 - wq/wk columns are permuted on host into an "even dims block / odd dims
   block" (A/B) layout so RoPE is full-partition DVE work; wq carries the
   1/sqrt(HD) scale (exact power of two).
 - Scores are computed transposed (scoresT[sk, sq]) so probsT feeds the AV
   matmul directly with no transposes in the attention path.
 - Causal mask: one extra accumulating matmul per diagonal tile,
   LEones[k,p]=[k<=p] x shifted -1e9 diagonal, adds -1e9 to masked entries.
 - Softmax denominators ride along as a ones column in v (M=65 AV matmul);
   normalization multiplies by the partition-broadcast reciprocal.
"""

import os
import sys
import functools

import numpy as np

if "/opt/trn_rl_repo" not in sys.path:
    sys.path.insert(0, "/opt/trn_rl_repo")

B, S, D = 2, 2048, 2048
H, KVH = 32, 8
HD = D // H            # 64
N_CORES = 8
GROUP = 4              # cores per batch group (tensor parallel width)
HPC = 8                # query heads per core
KVPC = 2               # kv heads per core
SQC = 512              # sq chunk (psum bank width in fp32)
PT = 128               # partition tile
KT = D // PT           # 16 contraction tiles
NT = S // PT           # 16 token tiles
NCHUNK = S // SQC      # 4
TPC = SQC // PT        # tok tiles per chunk (4)
MASK_W = 896           # width of the shifted-diagonal mask table
NEG = -1e9


def _build_program(mm_dtype_name="float32r"):
    import concourse.bass as bass
    import concourse.bacc as bacc
    import concourse.mybir as mybir
    import concourse.tile as tile
    import ml_dtypes
    from contextlib import ExitStack

    f32 = mybir.dt.float32
    bf16 = mybir.dt.bfloat16
    mmdt = getattr(mybir.dt, mm_dtype_name)

    def mc(ap):  # bitcast for fp32r matmul operands/producers
        return ap.bitcast(mmdt) if mmdt != f32 else ap

    nc = bacc.Bacc("TRN2", target_bir_lowering=False, debug=False,
                   num_devices=N_CORES)

    # ---- dram parameters -------------------------------------------------
    xT_d = nc.dram_tensor("xt", [NCHUNK, D, SQC], bf16, kind="ExternalInput")
    wq_d = nc.dram_tensor("wq", [D, HPC * HD], bf16, kind="ExternalInput")
    wk_d = nc.dram_tensor("wk", [D, KVPC * HD], bf16, kind="ExternalInput")
    wv_d = nc.dram_tensor("wv", [D, KVPC * HD], bf16, kind="ExternalInput")
    wo_d = nc.dram_tensor("wo", [HPC * HD, D], f32, kind="ExternalInput")
    cos_d = nc.dram_tensor("cosr", [PT, S], bf16, kind="ExternalInput")
    sin_d = nc.dram_tensor("sinr", [PT, S], bf16, kind="ExternalInput")
    y_out = nc.dram_tensor("y", [S // GROUP, D], f32, kind="ExternalOutput")

    y_part = nc.dram_tensor("y_part", [S, D], f32)
    y_rs = nc.dram_tensor("y_rs", [S // GROUP, D], f32)

    # ---- inline constants ------------------------------------------------
    leones = np.zeros((PT, PT), np.float32)      # leones[k, p] = 1 if k <= p
    for k in range(PT):
        leones[k, k:] = 1.0
    dmaster = np.zeros((PT, MASK_W), np.float32)
    # col m: -1e9 one-hot at k = m-383 for m in [384, 510]; row 0 for m<384
    dmaster[0, :384] = NEG
    for m in range(384, 511):
        dmaster[m - 383, m] = NEG
    ident = np.eye(PT, dtype=ml_dtypes.bfloat16)
    ones1 = np.ones((PT, 1), ml_dtypes.bfloat16)

    le_d = nc.inline_tensor(leones.astype(ml_dtypes.bfloat16), "leones")
    dm_d = nc.inline_tensor(dmaster.astype(ml_dtypes.bfloat16), "dmaster")
    id_d = nc.inline_tensor(ident, "ident")
    on_d = nc.inline_tensor(ones1, "ones1")

    Exp = mybir.ActivationFunctionType.Exp
    groups = [[0, 1, 2, 3], [4, 5, 6, 7]]

    with tile.TileContext(nc) as tc, ExitStack() as ctx:
        keep = ctx.enter_context(tc.tile_pool(name="keep", bufs=1))
        # packed K cache: krp[kv] rows = [kv(a32 b32); kv(a32 b32)] replicated
        krp0 = keep.tile([PT, S], bf16)
        krp1 = keep.tile([PT, S], bf16)
        krp = [krp0, krp1]
        v_sb = keep.tile([PT, KVPC, NT, HD + 1], bf16)   # col 64 = ones
        cos_sb = keep.tile([PT, S], bf16)
        sin_sb = keep.tile([PT, S], bf16)
        le_sb = keep.tile([PT, PT], bf16)
        dm_sb = keep.tile([PT, MASK_W], bf16)
        id_sb = keep.tile([PT, PT], bf16)
        wq_sb = keep.tile([PT, KT, HPC * HD], bf16)
        wk_sb = keep.tile([PT, KT, KVPC * HD], bf16)
        wv_sb = keep.tile([PT, KT, KVPC * HD], bf16)
        wo_sb = keep.tile([PT, 4, D], f32)

        xcache = {}
        qcps = {}

        def load_x(c):
            halves = []
            for hf in range(2):
                xt = xpool.tile([PT, KT // 2, SQC], bf16, tag="xt",
                                name=f"xt{c}_{hf}")
                nc.sync.dma_start(
                    out=xt[:],
                    in_=xT_d[c].rearrange("(k p) n -> p k n", p=PT)
                    [:, hf * (KT // 2):(hf + 1) * (KT // 2), :])
                halves.append(xt)
            xcache[c] = halves

        nc.sync.dma_start(out=wq_sb[:],
                          in_=wq_d.ap().rearrange("(k p) n -> p k n", p=PT))
        nc.sync.dma_start(out=wk_sb[:],
                          in_=wk_d.ap().rearrange("(k p) n -> p k n", p=PT))
        nc.sync.dma_start(out=wv_sb[:],
                          in_=wv_d.ap().rearrange("(k p) n -> p k n", p=PT))

        xpool = ctx.enter_context(tc.tile_pool(name="xp", bufs=4))
        qpool = ctx.enter_context(tc.tile_pool(name="qp", bufs=2))
        qppool = ctx.enter_context(tc.tile_pool(name="qpp", bufs=2))
        kpool = ctx.enter_context(tc.tile_pool(name="kp", bufs=2))
        vtp = ctx.enter_context(tc.tile_pool(name="vtp", bufs=2))
        otp = ctx.enter_context(tc.tile_pool(name="otp", bufs=2))
        rtmp = ctx.enter_context(tc.tile_pool(name="rtmp", bufs=1))
        probs = ctx.enter_context(tc.tile_pool(name="probs", bufs=8))
        bcp = ctx.enter_context(tc.tile_pool(name="bcp", bufs=2))
        rcp = ctx.enter_context(tc.tile_pool(name="rcp", bufs=2))
        osg = ctx.enter_context(tc.tile_pool(name="osg", bufs=2))
        ysb = ctx.enter_context(tc.tile_pool(name="ysb", bufs=3))
        mw = ctx.enter_context(tc.tile_pool(name="mw", bufs=2, space="PSUM"))
        sps = ctx.enter_context(tc.tile_pool(name="sps", bufs=4, space="PSUM"))
        aps = ctx.enter_context(tc.tile_pool(name="aps", bufs=2, space="PSUM"))

        load_x(0)
        nc.sync.dma_start(out=cos_sb[:], in_=cos_d[:])
        nc.sync.dma_start(out=sin_sb[:], in_=sin_d[:])
        nc.sync.dma_start(out=le_sb[:], in_=le_d[:])
        nc.sync.dma_start(out=dm_sb[:], in_=dm_d[:])
        nc.sync.dma_start(out=id_sb[:], in_=id_d[:])
        # ones column of v (every (kv, t) slot)
        ones_src = bass.AP(tensor=on_d.ap().tensor, offset=0,
                           ap=[[1, PT], [0, KVPC * NT], [1, 1]])
        vcol = v_sb[:, :, :, HD:HD + 1]
        ones_dst = bass.AP(tensor=vcol.tensor, offset=vcol.offset,
                           ap=[list(vcol.ap[0]), [HD + 1, KVPC * NT], [1, 1]])
        nc.sync.dma_start(out=ones_dst, in_=ones_src)
        nc.sync.dma_start(out=mc(wo_sb[:]),
                          in_=mc(wo_d.ap().rearrange("(k p) n -> p k n", p=PT)))

        def rope_pair(a, b, cs, sn, nm):
            """a' = a*cos - b*sin ; b' = a*sin + b*cos (bf16, in place)."""
            t1 = rtmp.tile(a.shape, bf16, tag="t1", name=f"t1{nm}")
            t2 = rtmp.tile(a.shape, bf16, tag="t2", name=f"t2{nm}")
            t3 = rtmp.tile(a.shape, bf16, tag="t3", name=f"t3{nm}")
            nc.vector.tensor_mul(t1[:], a, cs)
            nc.vector.tensor_mul(t2[:], a, sn)
            nc.vector.tensor_mul(t3[:], b, sn)
            nc.vector.tensor_sub(a, t1[:], t3[:])
            t4 = rtmp.tile(a.shape, bf16, tag="t3", name=f"t4{nm}")
            nc.vector.tensor_mul(t4[:], b, cs)
            nc.vector.tensor_add(b, t2[:], t4[:])

        def prep(c):
            csl = slice(c * SQC, (c + 1) * SQC)
            if c not in xcache:
                load_x(c)
            halves = xcache.pop(c)

            qc = qpool.tile([PT, 4, SQC], bf16, tag="qc", name=f"qc{c}")
            kc = kpool.tile([PT, SQC], bf16, tag="kc", name=f"kc{c}")
            vtc = vtp.tile([PT, SQC], bf16, tag="vtc", name=f"vtc{c}")
            for mt in range(4):
                ps = mw.tile([PT, SQC], f32, tag="ps", name=f"qps{c}_{mt}")
                for k in range(KT):
                    nc.tensor.matmul(
                        ps[:], wq_sb[:, k, mt * PT:(mt + 1) * PT],
                        halves[k // 8][:, k % 8, :],
                        start=(k == 0), stop=(k == KT - 1))
                nc.scalar.copy(qc[:, mt, :], ps[:])
            for dst, wsb, nm in ((kc, wk_sb, "k"), (vtc, wv_sb, "v")):
                ps = mw.tile([PT, SQC], f32, tag="ps", name=f"ps{nm}{c}")
                for k in range(KT):
                    nc.tensor.matmul(
                        ps[:], wsb[:, k, :],
                        halves[k // 8][:, k % 8, :],
                        start=(k == 0), stop=(k == KT - 1))
                nc.scalar.copy(dst[:], ps[:])

            # ---- rope(c) ------------------------------------------------
            for j in range(2):
                rope_pair(qc[:, j, :], qc[:, 2 + j, :],
                          cos_sb[:, csl], sin_sb[:, csl], f"q{c}_{j}")
            # k pair: rows 0:64 / 64:128 — stage B rows to base 0 via DMA
            bst = rtmp.tile([64, SQC], bf16, tag="t1", name=f"bst{c}")
            nc.sync.dma_start(out=bst[:], in_=kc[64:128, :])
            kt1 = rtmp.tile([64, SQC], bf16, tag="t2", name=f"kt1{c}")
            kt2 = rtmp.tile([64, SQC], bf16, tag="t3", name=f"kt2{c}")
            kt3 = rtmp.tile([64, SQC], bf16, tag="t1b", name=f"kt3{c}")
            kt4 = rtmp.tile([64, SQC], bf16, tag="t2b", name=f"kt4{c}")
            nc.vector.tensor_mul(kt1[:], kc[0:64, :], cos_sb[0:64, csl])
            nc.vector.tensor_mul(kt2[:], kc[0:64, :], sin_sb[0:64, csl])
            nc.vector.tensor_mul(kt3[:], bst[:], sin_sb[0:64, csl])
            nc.vector.tensor_mul(kt4[:], bst[:], cos_sb[0:64, csl])
            nc.vector.tensor_sub(kc[0:64, :], kt1[:], kt3[:])
            kbr = rtmp.tile([64, SQC], bf16, tag="t3b", name=f"kbr{c}")
            nc.vector.tensor_add(kbr[:], kt2[:], kt4[:])
            nc.sync.dma_start(out=kc[64:128, :], in_=kbr[:])

            # ---- pack(c): head-contiguous q (K=64 scores) ---------------
            # qcp[j] rows: head 2j at [0:64] (a32 b32), head 2j+1 at [64:128]
            qcp = qppool.tile([PT, 4, SQC], bf16, tag="qcp", name=f"qcp{c}")
            qcps[c] = qcp
            for qh in range(HPC):
                so = slice((qh % 4) * 32, (qh % 4) * 32 + 32)
                do = (qh % 2) * 64
                nc.sync.dma_start(out=qcp[do:do + 32, qh // 2, :],
                                  in_=qc[so, qh // 4, :])
                nc.sync.dma_start(out=qcp[do + 32:do + 64, qh // 2, :],
                                  in_=qc[so, 2 + qh // 4, :])
            # krp[kv] rows [0:64]=[64:128] = kv's (a32 b32)
            for kv in range(KVPC):
                for rep in range(2):
                    ro = rep * 64
                    nc.sync.dma_start(
                        out=krp[kv][ro:ro + 32, csl],
                        in_=kc[kv * 32:(kv + 1) * 32, :])
                    nc.sync.dma_start(
                        out=krp[kv][ro + 32:ro + 64, csl],
                        in_=kc[64 + kv * 32:64 + (kv + 1) * 32, :])

            # ---- v(c): transpose vT chunk into v_sb ---------------------
            for tl in range(TPC):
                t = c * TPC + tl
                tp = mw.tile([PT, SQC], f32, tag="ps", name=f"tp{c}_{tl}")
                tpb = tp[:, 0:PT].bitcast(bf16)[:, 0:PT]
                nc.tensor.transpose(tpb,
                                    vtc[:, tl * PT:(tl + 1) * PT],
                                    id_sb[:])
                nc.vector.tensor_copy(v_sb[:, 0, t, 0:HD], tpb[:, 0:HD])
                nc.vector.tensor_copy(v_sb[:, 1, t, 0:HD], tpb[:, HD:2 * HD])


        prep(0)
        for c in range(NCHUNK):
            csl = slice(c * SQC, (c + 1) * SQC)
            if c + 1 < NCHUNK:
                prep(c + 1)
            # ---- attention(c) -------------------------------------------
            qcp = qcps.pop(c)
            outc = otp.tile([PT, 4, SQC], f32, tag="outc", name=f"outc{c}")
            ntk = 4 * c + 4
            LAG = 2
            for qh in range(HPC):
                    g = qh // 4            # kv group
                    base = (qh % 2) * 64
                    av = aps.tile([PT, SQC], f32, tag="av",
                                  name=f"av{c}_{qh}")
                    pbq = []
                    for tt in range(ntk + LAG):
                        if tt < ntk:
                            t = tt
                            ksl = slice(t * PT, (t + 1) * PT)
                            sc = sps.tile([PT, SQC], f32, tag="sc",
                                          name=f"sc{c}_{qh}_{t}")
                            diag = t >= 4 * c
                            nc.tensor.matmul(
                                sc[:], krp[g][base:base + 64, ksl],
                                qcp[base:base + 64, qh // 2, :],
                                start=True, stop=not diag,
                                tile_position=(base, 0))
                            if diag:
                                r = t - 4 * c
                                nc.tensor.matmul(
                                    sc[:], le_sb[:],
                                    dm_sb[:, 384 - 128 * r:MASK_W - 128 * r],
                                    start=False, stop=True)
                            pb = probs.tile([PT, SQC], bf16, tag="pb",
                                            name=f"pb{c}_{qh}_{t}")
                            nc.scalar.activation(pb[:], sc[:], Exp)
                            pbq.append(pb)
                        if tt >= LAG:
                            t = tt - LAG
                            nc.tensor.matmul(
                                av[0:HD + 1, :], v_sb[:, g, t, :],
                                pbq[t][:],
                                start=(t == 0), stop=(t == ntk - 1))
                    rc = rcp.tile([1, SQC], f32, tag="rc",
                                  name=f"rc{c}_{qh}")
                    lg = rcp.tile([1, SQC], f32, tag="lg",
                                  name=f"lg{c}_{qh}")
                    nc.scalar.activation(lg[:], av[HD:HD + 1, :],
                                         mybir.ActivationFunctionType.Ln)
                    nc.scalar.activation(rc[:], lg[:],
                                         mybir.ActivationFunctionType.Exp,
                                         scale=-1.0)
                    bc = bcp.tile([64, SQC], f32, tag="bc",
                                  name=f"bc{c}_{qh}")
                    nc.gpsimd.partition_broadcast(bc[:], rc[:])
                    dst = outc[(qh % 2) * HD:(qh % 2 + 1) * HD, qh // 2, :]
                    if qh % 2 == 0:
                        nc.vector.tensor_mul(mc(dst), av[0:HD, :], bc[:])
                    else:
                        st = osg.tile([64, SQC], f32, tag="st",
                                      name=f"st{c}_{qh}")
                        nc.vector.tensor_mul(st[:], av[0:HD, :], bc[:])
                        nc.sync.dma_start(out=mc(dst), in_=mc(st[:]))

            # ---- wo(c) --------------------------------------------------
            for tl in range(TPC):
                tt = c * TPC + tl
                yt = ysb.tile([PT, D], f32, tag="yt", name=f"yt{c}_{tl}")
                for nk in range(4):
                    yp = mw.tile([PT, SQC], f32, tag="ps",
                                 name=f"yp{c}_{tl}_{nk}")
                    for k4 in range(4):
                        nc.tensor.matmul(
                            yp[:], mc(outc[:, k4, tl * PT:(tl + 1) * PT]),
                            mc(wo_sb[:, k4, nk * SQC:(nk + 1) * SQC]),
                            start=(k4 == 0), stop=(k4 == 3))
                    nc.vector.tensor_copy(yt[:, nk * SQC:(nk + 1) * SQC],
                                          yp[:])
                nc.sync.dma_start(out=y_part[tt * PT:(tt + 1) * PT, :],
                                  in_=yt[:])

            # ---- RS(c): reduce-scatter this chunk (last chunk split per
            #      tok-tile so the kernel tail is one small collective) ----
            if c > 0:
                # previous chunk's RS is long done; this wait is free
                nc.sync.dma_start(
                    out=y_out.ap()[(c - 1) * PT:c * PT, :],
                    in_=y_rs.ap()[(c - 1) * PT:c * PT, :])
            if c < NCHUNK - 1:
                nc.gpsimd.collective_compute(
                    "ReduceScatter", mybir.AluOpType.add,
                    replica_groups=groups,
                    ins=[y_part.ap()[csl, :]],
                    outs=[y_rs.ap()[c * PT:(c + 1) * PT, :]])
            else:
                q = PT // TPC   # 32 rows out per tok-tile RS
                for tl in range(TPC):
                    tt = c * TPC + tl
                    nc.gpsimd.collective_compute(
                        "ReduceScatter", mybir.AluOpType.add,
                        replica_groups=groups,
                        ins=[y_part.ap()[tt * PT:(tt + 1) * PT, :]],
                        outs=[y_rs.ap()[c * PT + tl * q:
                                        c * PT + (tl + 1) * q, :]])

        nc.sync.dma_start(
            out=y_out.ap()[(NCHUNK - 1) * PT:NCHUNK * PT, :],
            in_=y_rs.ap()[(NCHUNK - 1) * PT:NCHUNK * PT, :])

    nc.compile()
    return nc


@functools.lru_cache(maxsize=2)
def _get_program(mm_dtype_name="float32r"):
    return _build_program(mm_dtype_name)


def _host_inputs(x, wq, wk, wv, wo, cos, sin):
    """Build the 8 per-core input maps."""
    import ml_dtypes

    perm_q = np.empty(HPC * HD, np.int64)
    for rho in range(HPC * HD):
        blk, rem = divmod(rho, HPC * HD // 2)
        h, i = divmod(rem, 32)
        perm_q[rho] = h * HD + 2 * i + blk
    perm_k = np.empty(KVPC * HD, np.int64)
    for rho in range(KVPC * HD):
        blk, rem = divmod(rho, KVPC * HD // 2)
        kv, i = divmod(rem, 32)
        perm_k[rho] = kv * HD + 2 * i + blk

    reps = np.tile(np.arange(32), 4)
    cosr = np.ascontiguousarray(cos.T[reps]).astype(ml_dtypes.bfloat16)
    sinr = np.ascontiguousarray(sin.T[reps]).astype(ml_dtypes.bfloat16)

    xts = []
    for b in range(B):
        xt = x[b].T.reshape(D, NCHUNK, SQC)       # [D, 4, 512]
        xts.append(np.ascontiguousarray(xt.transpose(1, 0, 2))
                   .astype(ml_dtypes.bfloat16))

    scale = np.float32(1.0 / np.sqrt(HD))
    in_maps = []
    for core in range(N_CORES):
        b, hg = divmod(core, GROUP)
        qcols = slice(hg * HPC * HD, (hg + 1) * HPC * HD)
        kcols = slice(hg * KVPC * HD, (hg + 1) * KVPC * HD)
        wq_c = (wq[:, qcols] * scale)[:, perm_q].astype(ml_dtypes.bfloat16)
        wk_c = wk[:, kcols][:, perm_k].astype(ml_dtypes.bfloat16)
        wv_c = np.ascontiguousarray(wv[:, kcols]).astype(ml_dtypes.bfloat16)
        wo_c = np.ascontiguousarray(wo[qcols, :])
        in_maps.append({
            "xt": xts[b],
            "wq": np.ascontiguousarray(wq_c),
            "wk": np.ascontiguousarray(wk_c),
            "wv": wv_c,
            "wo": wo_c,
            "cosr": cosr,
            "sinr": sinr,
        })
    return in_maps


def _assemble(results):
    """results[core]["y"]: chunks 0..2 are [128]-row RS quarters; chunk 3
    was reduce-scattered per tok-tile ([32]-row quarters)."""
    out = np.empty((B, S, D), np.float32)
    q = PT // TPC
    for b in range(B):
        for r in range(GROUP):
            y = results[b * GROUP + r]["y"]
            for c in range(NCHUNK - 1):
                rows = slice(c * SQC + r * PT, c * SQC + (r + 1) * PT)
                out[b, rows, :] = y[c * PT:(c + 1) * PT, :]
            c = NCHUNK - 1
            for tl in range(TPC):
                tt = c * TPC + tl
                rows = slice(tt * PT + r * q, tt * PT + (r + 1) * q)
                out[b, rows, :] = y[c * PT + tl * q:c * PT + (tl + 1) * q, :]
    return out


def _is_causal(mask):
    if mask.shape != (S, S):
        return False
    expect = np.where(np.tril(np.ones((S, S), bool)), np.float32(0.0),
                      np.float32(NEG))
    return np.array_equal(mask, expect)


def _numpy_fallback(x, wq, wk, wv, wo, cos, sin, mask):
    """Exact reference math on host (only used if mask isn't causal)."""
    xq = (x @ wq).reshape(B, S, H, HD)
    xk = (x @ wk).reshape(B, S, KVH, HD)
    xv = (x @ wv).reshape(B, S, KVH, HD)

    def rope(t):
        tr = t.reshape(*t.shape[:-1], HD // 2, 2)
        a, b = tr[..., 0], tr[..., 1]
        c = cos[None, :, None, :]
        s_ = sin[None, :, None, :]
        out = np.stack([a * c - b * s_, a * s_ + b * c], axis=-1)
        return out.reshape(t.shape)

    xq, xk = rope(xq), rope(xk)
    xk = np.repeat(xk, H // KVH, axis=2)
    xv = np.repeat(xv, H // KVH, axis=2)
    q = xq.transpose(0, 2, 1, 3)
    k = xk.transpose(0, 2, 1, 3)
    v = xv.transpose(0, 2, 1, 3)
    sc = np.einsum("bhqd,bhkd->bhqk", q, k) / np.sqrt(np.float32(HD))
    sc = sc + mask[None, None]
    sc = sc - sc.max(-1, keepdims=True)
    p = np.exp(sc)
    p /= p.sum(-1, keepdims=True)
    out = np.einsum("bhqk,bhkd->bhqd", p, v)
    out = out.transpose(0, 2, 1, 3).reshape(B, S, H * HD)
    return (out @ wo).astype(np.float32)


def _ensure_ntff_hook():
    """Provide antenv.axon_hooks (missing on this image) so trace=True works."""
    try:
        from antenv.axon_hooks import get_axon_ntff_profile_hook  # noqa: F401
        return True
    except ImportError:
        pass
    try:
        import types
        import antenv
        from trn_agent_boot.trn_boot import _ntff_profile_via_ctypes

        mod = types.ModuleType("antenv.axon_hooks")
        _state = {"hook": None}
        mod.set_axon_ntff_profile_hook = \
            lambda h: _state.__setitem__("hook", h)
        mod.get_axon_ntff_profile_hook = lambda: _state["hook"]
        sys.modules["antenv.axon_hooks"] = mod
        antenv.axon_hooks = mod
        mod.set_axon_ntff_profile_hook(
            _ntff_profile_via_ctypes("/opt/axon/libaxon_pjrt.so"))
        return mod.get_axon_ntff_profile_hook() is not None
    except Exception:
        return False


def kernel(x, wq, wk, wv, wo, cos, sin, mask):
    x = np.asarray(x, np.float32)
    wq = np.asarray(wq, np.float32)
    wk = np.asarray(wk, np.float32)
    wv = np.asarray(wv, np.float32)
    wo = np.asarray(wo, np.float32)
    cos = np.asarray(cos, np.float32)
    sin = np.asarray(sin, np.float32)
    mask = np.asarray(mask, np.float32)

    if not _is_causal(mask):
        return _numpy_fallback(x, wq, wk, wv, wo, cos, sin, mask)

    from concourse.bass_utils import run_bass_kernel_spmd

    nc = _get_program(os.environ.get("ATTN_MM_DTYPE", "float32r"))
    in_maps = _host_inputs(x, wq, wk, wv, wo, cos, sin)
    trace = bool(int(os.environ.get("ATTN_TRACE", "0")))
    if trace and not _ensure_ntff_hook():
        trace = False
    res = run_bass_kernel_spmd(nc, in_maps, core_ids=list(range(N_CORES)),
                               trace=trace)
    if trace:
        kernel.last_exec_time_ns = res.exec_time_ns
        kernel.last_results = res
    return _assemble(res.results)


# revision 25
# speedup vs baseline: 1.4360x; 1.0706x over previous
"""Trainium2 Bass kernel for GQA attention (B=2, S=2048, D=2048, H=32, KVH=8).

Sharding: batch data-parallel across 2 groups of 4 cores; within a group,
4-way tensor parallel over heads (8 q heads + their 2 kv heads per core).
Device-side per-chunk ReduceScatter(add) over each 4-core group after the wo
matmul; the host concatenates the token slices.

The device program is identical on all 8 cores (SPMD); all per-core
variation (batch slice, head slice) is carried by the input data.

v2 structure: a software pipeline over 512-token chunks —
   proj(c) -> rope(c) -> kTrep/v(c) -> attention(c) -> wo(c) -> RS(c)
so the tensor engine stays dense (HAM stays warm) and the collective
overlaps compute. Projections and the wo matmul run in fp32r; the
attention path (q/k/v/probs/cos/mask) runs in bf16 (fp32 PSUM accum).

Layout notes:
 - Host passes x pre-transposed (xT, chunk-major); every matmul consumes xT
   directly (no on-device transposes of activations).
 - wq/wk columns are permuted on host into an "even dims block / odd dims
   block" (A/B) layout so RoPE is full-partition DVE work; wq carries the
   1/sqrt(HD) scale (exact power of two).
 - Scores are computed transposed (scoresT[sk, sq]) so probsT feeds the AV
   matmul directly with no transposes in the attention path.
 - Causal mask: one extra accumulating matmul per diagonal tile,
   LEones[k,p]=[k<=p] x shifted -1e9 diagonal, adds -1e9 to masked entries.
 - Softmax denominators ride along as a ones column in v (M=65 AV matmul);
   normalization multiplies by the partition-broadcast reciprocal.
"""

import os
import sys
import functools

import numpy as np

if "/opt/trn_rl_repo" not in sys.path:
    sys.path.insert(0, "/opt/trn_rl_repo")

B, S, D = 2, 2048, 2048
H, KVH = 32, 8
HD = D // H            # 64
N_CORES = 8
GROUP = 4              # cores per batch group (tensor parallel width)
HPC = 8                # query heads per core
KVPC = 2               # kv heads per core
SQC = 512              # sq chunk (psum bank width in fp32)
PT = 128               # partition tile
KT = D // PT           # 16 contraction tiles
NT = S // PT           # 16 token tiles
NCHUNK = S // SQC      # 4
TPC = SQC // PT        # tok tiles per chunk (4)
MASK_W = 896           # width of the shifted-diagonal mask table
NEG = -1e9


def _build_program(mm_dtype_name="float32r"):
    import concourse.bass as bass
    import concourse.bacc as bacc
    import concourse.mybir as mybir
    import concourse.tile as tile
    import ml_dtypes
    from contextlib import ExitStack

    f32 = mybir.dt.float32
    bf16 = mybir.dt.bfloat16
    mmdt = getattr(mybir.dt, mm_dtype_name)

    def mc(ap):  # bitcast for fp32r matmul operands/producers
        return ap.bitcast(mmdt) if mmdt != f32 else ap

    nc = bacc.Bacc("TRN2", target_bir_lowering=False, debug=False,
                   num_devices=N_CORES)

    # ---- dram parameters -------------------------------------------------
    xT_d = nc.dram_tensor("xt", [NCHUNK, D, SQC], bf16, kind="ExternalInput")
    wq_d = nc.dram_tensor("wq", [D, HPC * HD], bf16, kind="ExternalInput")
    wk_d = nc.dram_tensor("wk", [D, KVPC * HD], bf16, kind="ExternalInput")
    wv_d = nc.dram_tensor("wv", [D, KVPC * HD], bf16, kind="ExternalInput")
    wo_d = nc.dram_tensor("wo", [HPC * HD, D], f32, kind="ExternalInput")
    cos_d = nc.dram_tensor("cosr", [PT, S], bf16, kind="ExternalInput")
    sin_d = nc.dram_tensor("sinr", [PT, S], bf16, kind="ExternalInput")
    y_out = nc.dram_tensor("y", [S // GROUP, D], f32, kind="ExternalOutput")

    y_part = nc.dram_tensor("y_part", [S, D], f32)
    y_rs = nc.dram_tensor("y_rs", [S // GROUP, D], f32)

    # ---- inline constants ------------------------------------------------
    leones = np.zeros((PT, PT), np.float32)      # leones[k, p] = 1 if k <= p
    for k in range(PT):
        leones[k, k:] = 1.0
    dmaster = np.zeros((PT, MASK_W), np.float32)
    # col m: -1e9 one-hot at k = m-383 for m in [384, 510]; row 0 for m<384
    dmaster[0, :384] = NEG
    for m in range(384, 511):
        dmaster[m - 383, m] = NEG
    ident = np.eye(PT, dtype=ml_dtypes.bfloat16)
    ones1 = np.ones((PT, 1), ml_dtypes.bfloat16)

    le_d = nc.inline_tensor(leones.astype(ml_dtypes.bfloat16), "leones")
    dm_d = nc.inline_tensor(dmaster.astype(ml_dtypes.bfloat16), "dmaster")
    id_d = nc.inline_tensor(ident, "ident")
    on_d = nc.inline_tensor(ones1, "ones1")

    Exp = mybir.ActivationFunctionType.Exp
    groups = [[0, 1, 2, 3], [4, 5, 6, 7]]

    with tile.TileContext(nc) as tc, ExitStack() as ctx:
        keep = ctx.enter_context(tc.tile_pool(name="keep", bufs=1))
        # packed K cache: krp[kv] rows = [kv(a32 b32); kv(a32 b32)] replicated
        krp0 = keep.tile([PT, S], bf16)
        krp1 = keep.tile([PT, S], bf16)
        krp = [krp0, krp1]
        v_sb = keep.tile([PT, KVPC, NT, HD + 1], bf16)   # col 64 = ones
        cos_sb = keep.tile([PT, S], bf16)
        sin_sb = keep.tile([PT, S], bf16)
        le_sb = keep.tile([PT, PT], bf16)
        dm_sb = keep.tile([PT, MASK_W], bf16)
        id_sb = keep.tile([PT, PT], bf16)
        wq_sb = keep.tile([PT, KT, HPC * HD], bf16)
        wk_sb = keep.tile([PT, KT, KVPC * HD], bf16)
        wv_sb = keep.tile([PT, KT, KVPC * HD], bf16)
        wo_sb = keep.tile([PT, 4, D], f32)

        xcache = {}
        qcps = {}

        def load_x(c):
            halves = []
            for hf in range(2):
                xt = xpool.tile([PT, KT // 2, SQC], bf16, tag="xt",
                                name=f"xt{c}_{hf}")
                nc.sync.dma_start(
                    out=xt[:],
                    in_=xT_d[c].rearrange("(k p) n -> p k n", p=PT)
                    [:, hf * (KT // 2):(hf + 1) * (KT // 2), :])
                halves.append(xt)
            xcache[c] = halves

        nc.sync.dma_start(out=wq_sb[:],
                          in_=wq_d.ap().rearrange("(k p) n -> p k n", p=PT))
        nc.sync.dma_start(out=wk_sb[:],
                          in_=wk_d.ap().rearrange("(k p) n -> p k n", p=PT))
        nc.sync.dma_start(out=wv_sb[:],
                          in_=wv_d.ap().rearrange("(k p) n -> p k n", p=PT))

        xpool = ctx.enter_context(tc.tile_pool(name="xp", bufs=4))
        qpool = ctx.enter_context(tc.tile_pool(name="qp", bufs=2))
        qppool = ctx.enter_context(tc.tile_pool(name="qpp", bufs=2))
        kpool = ctx.enter_context(tc.tile_pool(name="kp", bufs=2))
        vtp = ctx.enter_context(tc.tile_pool(name="vtp", bufs=2))
        otp = ctx.enter_context(tc.tile_pool(name="otp", bufs=2))
        rtmp = ctx.enter_context(tc.tile_pool(name="rtmp", bufs=1))
        probs = ctx.enter_context(tc.tile_pool(name="probs", bufs=8))
        bcp = ctx.enter_context(tc.tile_pool(name="bcp", bufs=2))
        rcp = ctx.enter_context(tc.tile_pool(name="rcp", bufs=2))
        osg = ctx.enter_context(tc.tile_pool(name="osg", bufs=2))
        ysb = ctx.enter_context(tc.tile_pool(name="ysb", bufs=3))
        mw = ctx.enter_context(tc.tile_pool(name="mw", bufs=2, space="PSUM"))
        sps = ctx.enter_context(tc.tile_pool(name="sps", bufs=4, space="PSUM"))
        aps = ctx.enter_context(tc.tile_pool(name="aps", bufs=2, space="PSUM"))

        load_x(0)
        nc.sync.dma_start(out=cos_sb[:], in_=cos_d[:])
        nc.sync.dma_start(out=sin_sb[:], in_=sin_d[:])
        nc.sync.dma_start(out=le_sb[:], in_=le_d[:])
        nc.sync.dma_start(out=dm_sb[:], in_=dm_d[:])
        nc.sync.dma_start(out=id_sb[:], in_=id_d[:])
        # ones column of v (every (kv, t) slot)
        ones_src = bass.AP(tensor=on_d.ap().tensor, offset=0,
                           ap=[[1, PT], [0, KVPC * NT], [1, 1]])
        vcol = v_sb[:, :, :, HD:HD + 1]
        ones_dst = bass.AP(tensor=vcol.tensor, offset=vcol.offset,
                           ap=[list(vcol.ap[0]), [HD + 1, KVPC * NT], [1, 1]])
        nc.sync.dma_start(out=ones_dst, in_=ones_src)

        def rope_pair(a, b, cs, sn, nm):
            """a' = a*cos - b*sin ; b' = a*sin + b*cos (bf16, in place)."""
            t1 = rtmp.tile(a.shape, bf16, tag="t1", name=f"t1{nm}")
            t2 = rtmp.tile(a.shape, bf16, tag="t2", name=f"t2{nm}")
            t3 = rtmp.tile(a.shape, bf16, tag="t3", name=f"t3{nm}")
            nc.vector.tensor_mul(t1[:], a, cs)
            nc.vector.tensor_mul(t2[:], a, sn)
            nc.vector.tensor_mul(t3[:], b, sn)
            nc.vector.tensor_sub(a, t1[:], t3[:])
            t4 = rtmp.tile(a.shape, bf16, tag="t3", name=f"t4{nm}")
            nc.vector.tensor_mul(t4[:], b, cs)
            nc.vector.tensor_add(b, t2[:], t4[:])

        def prep(c):
            csl = slice(c * SQC, (c + 1) * SQC)
            if c not in xcache:
                load_x(c)
            halves = xcache.pop(c)

            qc = qpool.tile([PT, 4, SQC], bf16, tag="qc", name=f"qc{c}")
            kc = kpool.tile([PT, SQC], bf16, tag="kc", name=f"kc{c}")
            vtc = vtp.tile([PT, SQC], bf16, tag="vtc", name=f"vtc{c}")
            for mt in range(4):
                ps = mw.tile([PT, SQC], f32, tag="ps", name=f"qps{c}_{mt}")
                for k in range(KT):
                    nc.tensor.matmul(
                        ps[:], wq_sb[:, k, mt * PT:(mt + 1) * PT],
                        halves[k // 8][:, k % 8, :],
                        start=(k == 0), stop=(k == KT - 1))
                nc.scalar.copy(qc[:, mt, :], ps[:])
            for dst, wsb, nm in ((kc, wk_sb, "k"), (vtc, wv_sb, "v")):
                ps = mw.tile([PT, SQC], f32, tag="ps", name=f"ps{nm}{c}")
                for k in range(KT):
                    nc.tensor.matmul(
                        ps[:], wsb[:, k, :],
                        halves[k // 8][:, k % 8, :],
                        start=(k == 0), stop=(k == KT - 1))
                nc.scalar.copy(dst[:], ps[:])

            # ---- rope(c) ------------------------------------------------
            for j in range(2):
                rope_pair(qc[:, j, :], qc[:, 2 + j, :],
                          cos_sb[:, csl], sin_sb[:, csl], f"q{c}_{j}")
            # k pair: rows 0:64 / 64:128 — stage B rows to base 0 via DMA
            bst = rtmp.tile([64, SQC], bf16, tag="t1", name=f"bst{c}")
            nc.sync.dma_start(out=bst[:], in_=kc[64:128, :])
            kt1 = rtmp.tile([64, SQC], bf16, tag="t2", name=f"kt1{c}")
            kt2 = rtmp.tile([64, SQC], bf16, tag="t3", name=f"kt2{c}")
            kt3 = rtmp.tile([64, SQC], bf16, tag="t1b", name=f"kt3{c}")
            kt4 = rtmp.tile([64, SQC], bf16, tag="t2b", name=f"kt4{c}")
            nc.vector.tensor_mul(kt1[:], kc[0:64, :], cos_sb[0:64, csl])
            nc.vector.tensor_mul(kt2[:], kc[0:64, :], sin_sb[0:64, csl])
            nc.vector.tensor_mul(kt3[:], bst[:], sin_sb[0:64, csl])
            nc.vector.tensor_mul(kt4[:], bst[:], cos_sb[0:64, csl])
            nc.vector.tensor_sub(kc[0:64, :], kt1[:], kt3[:])
            kbr = rtmp.tile([64, SQC], bf16, tag="t3b", name=f"kbr{c}")
            nc.vector.tensor_add(kbr[:], kt2[:], kt4[:])
            nc.sync.dma_start(out=kc[64:128, :], in_=kbr[:])

            # ---- pack(c): head-contiguous q (K=64 scores) ---------------
            # qcp[j] rows: head 2j at [0:64] (a32 b32), head 2j+1 at [64:128]
            qcp = qppool.tile([PT, 4, SQC], bf16, tag="qcp", name=f"qcp{c}")
            qcps[c] = qcp
            for qh in range(HPC):
                so = slice((qh % 4) * 32, (qh % 4) * 32 + 32)
                do = (qh % 2) * 64
                nc.sync.dma_start(out=qcp[do:do + 32, qh // 2, :],
                                  in_=qc[so, qh // 4, :])
                nc.sync.dma_start(out=qcp[do + 32:do + 64, qh // 2, :],
                                  in_=qc[so, 2 + qh // 4, :])
            # krp[kv] rows [0:64]=[64:128] = kv's (a32 b32)
            for kv in range(KVPC):
                for rep in range(2):
                    ro = rep * 64
                    nc.sync.dma_start(
                        out=krp[kv][ro:ro + 32, csl],
                        in_=kc[kv * 32:(kv + 1) * 32, :])
                    nc.sync.dma_start(
                        out=krp[kv][ro + 32:ro + 64, csl],
                        in_=kc[64 + kv * 32:64 + (kv + 1) * 32, :])

            # ---- v(c): transpose vT chunk into v_sb ---------------------
            for tl in range(TPC):
                t = c * TPC + tl
                tp = mw.tile([PT, SQC], f32, tag="ps", name=f"tp{c}_{tl}")
                tpb = tp[:, 0:PT].bitcast(bf16)[:, 0:PT]
                nc.tensor.transpose(tpb,
                                    vtc[:, tl * PT:(tl + 1) * PT],
                                    id_sb[:])
                nc.vector.tensor_copy(v_sb[:, 0, t, 0:HD], tpb[:, 0:HD])
                nc.vector.tensor_copy(v_sb[:, 1, t, 0:HD], tpb[:, HD:2 * HD])


        prep(0)
        for c in range(NCHUNK):
            csl = slice(c * SQC, (c + 1) * SQC)
            if c + 1 < NCHUNK:
                prep(c + 1)
            if c == 0:
                nc.sync.dma_start(
                    out=mc(wo_sb[:]),
                    in_=mc(wo_d.ap().rearrange("(k p) n -> p k n", p=PT)))

            # ---- attention(c) -------------------------------------------
            qcp = qcps.pop(c)
            outc = otp.tile([PT, 4, SQC], f32, tag="outc", name=f"outc{c}")
            ntk = 4 * c + 4
            LAG = 2
            for qh in range(HPC):
                    g = qh // 4            # kv group
                    base = (qh % 2) * 64
                    av = aps.tile([PT, SQC], f32, tag="av",
                                  name=f"av{c}_{qh}")
                    pbq = []
                    for tt in range(ntk + LAG):
                        if tt < ntk:
                            t = tt
                            ksl = slice(t * PT, (t + 1) * PT)
                            sc = sps.tile([PT, SQC], f32, tag="sc",
                                          name=f"sc{c}_{qh}_{t}")
                            diag = t >= 4 * c
                            nc.tensor.matmul(
                                sc[:], krp[g][base:base + 64, ksl],
                                qcp[base:base + 64, qh // 2, :],
                                start=True, stop=not diag,
                                tile_position=(base, 0))
                            if diag:
                                r = t - 4 * c
                                nc.tensor.matmul(
                                    sc[:], le_sb[:],
                                    dm_sb[:, 384 - 128 * r:MASK_W - 128 * r],
                                    start=False, stop=True)
                            pb = probs.tile([PT, SQC], bf16, tag="pb",
                                            name=f"pb{c}_{qh}_{t}")
                            nc.scalar.activation(pb[:], sc[:], Exp)
                            pbq.append(pb)
                        if tt >= LAG:
                            t = tt - LAG
                            nc.tensor.matmul(
                                av[0:HD + 1, :], v_sb[:, g, t, :],
                                pbq[t][:],
                                start=(t == 0), stop=(t == ntk - 1))
                    rc = rcp.tile([1, SQC], f32, tag="rc",
                                  name=f"rc{c}_{qh}")
                    nc.vector.reciprocal(rc[:], av[HD:HD + 1, :])
                    bc = bcp.tile([64, SQC], f32, tag="bc",
                                  name=f"bc{c}_{qh}")
                    nc.gpsimd.partition_broadcast(bc[:], rc[:])
                    dst = outc[(qh % 2) * HD:(qh % 2 + 1) * HD, qh // 2, :]
                    if qh % 2 == 0:
                        nc.vector.tensor_mul(mc(dst), av[0:HD, :], bc[:])
                    else:
                        st = osg.tile([64, SQC], f32, tag="st",
                                      name=f"st{c}_{qh}")
                        nc.vector.tensor_mul(st[:], av[0:HD, :], bc[:])
                        nc.sync.dma_start(out=mc(dst), in_=mc(st[:]))

            # ---- wo(c) --------------------------------------------------
            for tl in range(TPC):
                tt = c * TPC + tl
                yt = ysb.tile([PT, D], f32, tag="yt", name=f"yt{c}_{tl}")
                for nk in range(4):
                    yp = mw.tile([PT, SQC], f32, tag="ps",
                                 name=f"yp{c}_{tl}_{nk}")
                    for k4 in range(4):
                        nc.tensor.matmul(
                            yp[:], mc(outc[:, k4, tl * PT:(tl + 1) * PT]),
                            mc(wo_sb[:, k4, nk * SQC:(nk + 1) * SQC]),
                            start=(k4 == 0), stop=(k4 == 3))
                    nc.vector.tensor_copy(yt[:, nk * SQC:(nk + 1) * SQC],
                                          yp[:])
                nc.sync.dma_start(out=y_part[tt * PT:(tt + 1) * PT, :],
                                  in_=yt[:])
                if c == NCHUNK - 1:
                    q = PT // TPC
                    nc.gpsimd.collective_compute(
                        "ReduceScatter", mybir.AluOpType.add,
                        replica_groups=groups,
                        ins=[y_part.ap()[tt * PT:(tt + 1) * PT, :]],
                        outs=[y_rs.ap()[c * PT + tl * q:
                                        c * PT + (tl + 1) * q, :]])

            # ---- RS(c): reduce-scatter this chunk (last chunk split per
            #      tok-tile so the kernel tail is one small collective) ----
            if c > 0:
                # previous chunk's RS is long done; this wait is free
                nc.sync.dma_start(
                    out=y_out.ap()[(c - 1) * PT:c * PT, :],
                    in_=y_rs.ap()[(c - 1) * PT:c * PT, :])
            if c < NCHUNK - 1:
                nc.gpsimd.collective_compute(
                    "ReduceScatter", mybir.AluOpType.add,
                    replica_groups=groups,
                    ins=[y_part.ap()[csl, :]],
                    outs=[y_rs.ap()[c * PT:(c + 1) * PT, :]])

        nc.sync.dma_start(
            out=y_out.ap()[(NCHUNK - 1) * PT:NCHUNK * PT, :],
            in_=y_rs.ap()[(NCHUNK - 1) * PT:NCHUNK * PT, :])

    nc.compile()
    return nc


@functools.lru_cache(maxsize=2)
def _get_program(mm_dtype_name="float32r"):
    return _build_program(mm_dtype_name)


def _host_inputs(x, wq, wk, wv, wo, cos, sin):
    """Build the 8 per-core input maps."""
    import ml_dtypes

    perm_q = np.empty(HPC * HD, np.int64)
    for rho in range(HPC * HD):
        blk, rem = divmod(rho, HPC * HD // 2)
        h, i = divmod(rem, 32)
        perm_q[rho] = h * HD + 2 * i + blk
    perm_k = np.empty(KVPC * HD, np.int64)
    for rho in range(KVPC * HD):
        blk, rem = divmod(rho, KVPC * HD // 2)
        kv, i = divmod(rem, 32)
        perm_k[rho] = kv * HD + 2 * i + blk

    reps = np.tile(np.arange(32), 4)
    cosr = np.ascontiguousarray(cos.T[reps]).astype(ml_dtypes.bfloat16)
    sinr = np.ascontiguousarray(sin.T[reps]).astype(ml_dtypes.bfloat16)

    xts = []
    for b in range(B):
        xt = x[b].T.reshape(D, NCHUNK, SQC)       # [D, 4, 512]
        xts.append(np.ascontiguousarray(xt.transpose(1, 0, 2))
                   .astype(ml_dtypes.bfloat16))

    scale = np.float32(1.0 / np.sqrt(HD))
    in_maps = []
    for core in range(N_CORES):
        b, hg = divmod(core, GROUP)
        qcols = slice(hg * HPC * HD, (hg + 1) * HPC * HD)
        kcols = slice(hg * KVPC * HD, (hg + 1) * KVPC * HD)
        wq_c = (wq[:, qcols] * scale)[:, perm_q].astype(ml_dtypes.bfloat16)
        wk_c = wk[:, kcols][:, perm_k].astype(ml_dtypes.bfloat16)
        wv_c = np.ascontiguousarray(wv[:, kcols]).astype(ml_dtypes.bfloat16)
        wo_c = np.ascontiguousarray(wo[qcols, :])
        in_maps.append({
            "xt": xts[b],
            "wq": np.ascontiguousarray(wq_c),
            "wk": np.ascontiguousarray(wk_c),
            "wv": wv_c,
            "wo": wo_c,
            "cosr": cosr,
            "sinr": sinr,
        })
    return in_maps


def _assemble(results):
    """results[core]["y"]: chunks 0..2 are [128]-row RS quarters; chunk 3
    was reduce-scattered per tok-tile ([32]-row quarters)."""
    out = np.empty((B, S, D), np.float32)
    q = PT // TPC
    for b in range(B):
        for r in range(GROUP):
            y = results[b * GROUP + r]["y"]
            for c in range(NCHUNK - 1):
                rows = slice(c * SQC + r * PT, c * SQC + (r + 1) * PT)
                out[b, rows, :] = y[c * PT:(c + 1) * PT, :]
            c = NCHUNK - 1
            for tl in range(TPC):
                tt = c * TPC + tl
                rows = slice(tt * PT + r * q, tt * PT + (r + 1) * q)
                out[b, rows, :] = y[c * PT + tl * q:c * PT + (tl + 1) * q, :]
    return out


def _is_causal(mask):
    if mask.shape != (S, S):
        return False
    expect = np.where(np.tril(np.ones((S, S), bool)), np.float32(0.0),
                      np.float32(NEG))
    return np.array_equal(mask, expect)


def _numpy_fallback(x, wq, wk, wv, wo, cos, sin, mask):
    """Exact reference math on host (only used if mask isn't causal)."""
    xq = (x @ wq).reshape(B, S, H, HD)
    xk = (x @ wk).reshape(B, S, KVH, HD)
    xv = (x @ wv).reshape(B, S, KVH, HD)

    def rope(t):
        tr = t.reshape(*t.shape[:-1], HD // 2, 2)
        a, b = tr[..., 0], tr[..., 1]
        c = cos[None, :, None, :]
        s_ = sin[None, :, None, :]
        out = np.stack([a * c - b * s_, a * s_ + b * c], axis=-1)
        return out.reshape(t.shape)

    xq, xk = rope(xq), rope(xk)
    xk = np.repeat(xk, H // KVH, axis=2)
    xv = np.repeat(xv, H // KVH, axis=2)
    q = xq.transpose(0, 2, 1, 3)
    k = xk.transpose(0, 2, 1, 3)
    v = xv.transpose(0, 2, 1, 3)
    sc = np.einsum("bhqd,bhkd->bhqk", q, k) / np.sqrt(np.float32(HD))
    sc = sc + mask[None, None]
    sc = sc - sc.max(-1, keepdims=True)
    p = np.exp(sc)
    p /= p.sum(-1, keepdims=True)
    out = np.einsum("bhqk,bhkd->bhqd", p, v)
    out = out.transpose(0, 2, 1, 3).reshape(B, S, H * HD)
    return (out @ wo).astype(np.float32)


def _ensure_ntff_hook():
    """Provide antenv.axon_hooks (missing on this image) so trace=True works."""
    try:
        from antenv.axon_hooks import get_axon_ntff_profile_hook  # noqa: F401
        return True
    except ImportError:
        pass
    try:
        import types
        import antenv
        from trn_agent_boot.trn_boot import _ntff_profile_via_ctypes

        mod = types.ModuleType("antenv.axon_hooks")
        _state = {"hook": None}
        mod.set_axon_ntff_profile_hook = \
            lambda h: _state.__setitem__("hook", h)
        mod.get_axon_ntff_profile_hook = lambda: _state["hook"]
        sys.modules["antenv.axon_hooks"] = mod
        antenv.axon_hooks = mod
        mod.set_axon_ntff_profile_hook(
            _ntff_profile_via_ctypes("/opt/axon/libaxon_pjrt.so"))
        return mod.get_axon_ntff_profile_hook() is not None
    except Exception:
        return False


def kernel(x, wq, wk, wv, wo, cos, sin, mask):
    x = np.asarray(x, np.float32)
    wq = np.asarray(wq, np.float32)
    wk = np.asarray(wk, np.float32)
    wv = np.asarray(wv, np.float32)
    wo = np.asarray(wo, np.float32)
    cos = np.asarray(cos, np.float32)
    sin = np.asarray(sin, np.float32)
    mask = np.asarray(mask, np.float32)

    if not _is_causal(mask):
        return _numpy_fallback(x, wq, wk, wv, wo, cos, sin, mask)

    from concourse.bass_utils import run_bass_kernel_spmd

    nc = _get_program(os.environ.get("ATTN_MM_DTYPE", "float32r"))
    in_maps = _host_inputs(x, wq, wk, wv, wo, cos, sin)
    trace = bool(int(os.environ.get("ATTN_TRACE", "0")))
    if trace and not _ensure_ntff_hook():
        trace = False
    res = run_bass_kernel_spmd(nc, in_maps, core_ids=list(range(N_CORES)),
                               trace=trace)
    if trace:
        kernel.last_exec_time_ns = res.exec_time_ns
        kernel.last_results = res
    return _assemble(res.results)
